# revision 1
# baseline (speedup 1.0000x reference)
"""Trainium2 Bass kernel for nn_AttentionLayer (conv1d -> linear attention -> gelu + residual).

Full inputs:  x [8, 256, 4096] f32, conv_w [512, 256, 3] f32, conv_b [512] f32
Full output:  [8, 256, 4096] f32

Sharding: pure data-parallel over batch B=8 -> 8 NeuronCores, one batch each.
No collectives needed.

Per-core math (C=256, N=4096, one batch):
  y    = conv1d(x, w, pad=1) + b          # [2C, N]
  q    = phi(y[:C]),  k = phi(y[C:])      # phi = elu+1 = max(y+1, exp(min(y,0)))
  v    = x^T                              # [N, C]
  kv   = sum_n phi(k)[n,:] (x) v[n,:]     # [C, C]
  out  = gelu(q @ kv) + x                 # [C, N]

Layout trick: the conv contraction (over input channels ci) lets us produce
q in [c, n] layout (w^T as stationary operand) AND k in [n, c] layout
(x as stationary operand) with zero transposes. v^T (= x^T) is shipped
pre-transposed/pre-tiled from the host.

The conv matmuls (75% of FLOPs) run in fp8 E4M3 at 2x PE rate: weights are
host-scaled by 64 so they sit in E4M3's normal range (w in +-1/sqrt(768)
would be subnormal), and the 1/64 plus the "+1" of phi fold into the phi
chain for free:  with raw = 64*(conv + bias + 1) accumulated in PSUM
(bias+1 enters via a rank-1 bf16 start-matmul),
  phi = max(raw/64, exp(min(raw, 64)/64 - 1)).
End-to-end rel err with fp8 conv: 3.8e-3 (gate 2e-2; bf16 was 2.4e-3).
The attention matmuls (kv, q@kv) stay bf16: kv entries are ~100x larger
and get no averaging benefit. A full-precision bf16 copy of x feeds the
residual add and v^T.

DMA hygiene (the real-HW bottleneck): every HWDGE dma_start costs ~650ns of
serialized ring dispatch, and scattered patterns fragment into 512B
descriptors (the naive v^T load alone was ~0.5ms on HW). All inputs are
host-packed >=512B-per-partition contiguous and ordered so conv-critical
bytes land first; output stores are batched 1024-col bf16 pairs.

phi is 3 ops: min+scale (DVE) -> exp (ACT, one table per phase) -> scale
max e (DVE fused scalar_tensor_tensor). Residual add runs on DVE in bf16.
NT processes two n-tiles per PSUM bank with one phi chain.
"""

import contextlib

import ml_dtypes
import numpy as np

import concourse.bass as bass
import concourse.mybir as mybir
import concourse.tile as tile
from concourse import bacc
from concourse.bass_utils import run_bass_kernel_spmd

F32 = mybir.dt.float32
BF16 = mybir.dt.bfloat16
FP8 = mybir.dt.float8e4
AF = mybir.ActivationFunctionType
ALU = mybir.AluOpType

B, C, N = 8, 256, 4096
NCORES = 8
CT = C // 128        # 2 c-tiles (partition groups) per 256-channel dim
NJ = N // 512        # 8 column chunks of 512
NT = N // 128        # 32 n-tiles of 128
NP = N + 2           # x padded with one zero column on each side
NQ = 514             # leading x chunk so the conv starts ASAP
NH = NP // 2 + 1     # second-stage split point (2050)
WS = 64.0            # fp8 weight scale
NPP = 4112           # x8 row pitch: NP padded to 16B so the dual-fp8
                     # Ldweights outer stride is aligned

BF = ml_dtypes.bfloat16
F8 = ml_dtypes.float8_e4m3


def _build_nc(reps=1, hw_loop=False):
    nc = bacc.Bacc("TRN2", target_bir_lowering=False, debug=False, num_devices=NCORES)

    # Host-packed parameter layouts (see _prep):
    #  x8   [CT, 128, NP]   fp8   x padded (conv input)
    #  xb   [CT, 128, NP]   bf16  x padded (residual input)
    #  xt   [128, NT*256]   bf16  v^T tiled: [p, i*256+d] = x[d, i*128+p]
    #  w8k  [128, 6, 256]   fp8   conv w k-half x64; slot ci*3+t
    #  w8q  [128, 6, 256]   fp8   conv w q-half x64
    #  obk  [1, 1280]       bf16  [ones(512) | 64(bk+1) x2 (512) | 64(bq+1) (256)]
    x8_d = nc.declare_dram_parameter("x8", [128, CT * NPP], FP8, isOutput=False)
    x8s_d = nc.declare_dram_parameter("x8s", [128, CT * NPP], FP8, isOutput=False)
    xb_d = nc.declare_dram_parameter("xb", [CT, 128, NP], BF16, isOutput=False)
    xt_d = nc.declare_dram_parameter("xt", [128, NT * 256], BF16, isOutput=False)
    w8k_d = nc.declare_dram_parameter("w8k", [128, 6 * 256], FP8, isOutput=False)
    w8q_d = nc.declare_dram_parameter("w8q", [128, 6 * 256], FP8, isOutput=False)
    obk_d = nc.declare_dram_parameter("obk", [1, 1280], BF16, isOutput=False)
    out_d = nc.declare_dram_parameter("out", [C, N], BF16, isOutput=True)

    with tile.TileContext(nc) as tc:
        with (
            tc.tile_pool(name="persist", bufs=1) as per,
            tc.tile_pool(name="tmp", bufs=6) as tmp,
            tc.tile_pool(name="obuf", bufs=3) as obuf,
            tc.tile_pool(name="psum", bufs=6, space="PSUM") as ps,
            tc.tile_pool(name="psum2", bufs=2, space="PSUM") as ps2,
        ):
          loop_ctx = tc.For_i(0, reps, 1) if hw_loop else contextlib.nullcontext()
          with loop_ctx:
           for _rep in range(1 if hw_loop else reps):
            # ---- inputs: few large DMAs; conv-critical bytes first -------
            obk = per.tile([1, 1280], BF16, tag="obk", name="obk")
            nc.sync.dma_start(out=obk, in_=obk_d[:, :])
            ones128 = obk[0:1, 0:128]
            ones512 = obk[0:1, 0:512]
            bk2 = obk[0:1, 512:1024]
            # Warm the ACT Exp table while the bulk DMAs land (must read
            # initialized SBUF - a memzero'd scratch tile faults the exec
            # unit).
            warm = tmp.tile([1, 1], F32, tag="warm", name="warm")
            nc.scalar.activation(warm, obk[0:1, 0:1], AF.Exp)
            # exp bias const ln(64)-1: phi is stored x64 (undone by gelu's
            # input scale), which moves the clamp AFTER the exp:
            #   64*phi = max(raw, min(64*exp(z-1), 64))
            # so the DVE min op disappears from the phi chain entirely.
            bconst = per.tile([128, 1], F32, tag="bconst", name="bconst")
            nc.gpsimd.memset(bconst, 3.1588830833596715)

            w8k = per.tile([128, 3, 2, 256], FP8, tag="w8k", name="w8k")
            w8k_r = w8k_d.rearrange("p (t i c) -> p t i c", i=2, c=256)
            nc.sync.dma_start(out=w8k, in_=w8k_r)

            # x8s is x8 shifted left one column: the conv's t=1 tap needs
            # an odd byte offset, which dual-fp8 matmul operands disallow.
            x8 = per.tile([128, 2, NPP], FP8, tag="x8", name="x8")
            x8s = per.tile([128, 2, NPP], FP8, tag="x8s", name="x8s")
            x8_r = x8_d.rearrange("p (i n) -> p i n", n=NPP)
            x8s_r = x8s_d.rearrange("p (i n) -> p i n", n=NPP)
            nc.sync.dma_start(out=x8[:, :, 0:NQ], in_=x8_r[:, :, 0:NQ])
            nc.sync.dma_start(out=x8s[:, :, 0:NQ], in_=x8s_r[:, :, 0:NQ])
            NM = 1282
            nc.sync.dma_start(out=x8[:, :, NQ:NM], in_=x8_r[:, :, NQ:NM])
            nc.sync.dma_start(out=x8s[:, :, NQ:NM], in_=x8s_r[:, :, NQ:NM])
            nc.sync.dma_start(out=x8[:, :, NM:NH], in_=x8_r[:, :, NM:NH])
            nc.sync.dma_start(out=x8s[:, :, NM:NH], in_=x8s_r[:, :, NM:NH])
            NL = 3074
            nc.sync.dma_start(out=x8[:, :, NH:NL], in_=x8_r[:, :, NH:NL])
            nc.sync.dma_start(out=x8s[:, :, NH:NL], in_=x8s_r[:, :, NH:NL])
            nc.sync.dma_start(out=x8[:, :, NL:NPP], in_=x8_r[:, :, NL:NPP])
            nc.sync.dma_start(out=x8s[:, :, NL:NPP], in_=x8s_r[:, :, NL:NPP])

            w8q = per.tile([128, 3, 2, 256], FP8, tag="w8q", name="w8q")
            nc.sync.dma_start(
                out=w8q, in_=w8q_d.rearrange("p (t i c) -> p t i c", i=2, c=256))

            # v^T before the residual copy: the kv matmuls fused into NT
            # consume it from ~7us in, while xsb is idle until phase OUT.
            vT = per.tile([128, NT, 256], BF16, tag="vT", name="vT")
            vT_r = xt_d.rearrange("p (i d) -> p i d", d=256)
            # second half first: NT's fused kv matmuls (tiles 16-31) need it
            # before Q's (tiles 0-15, deferred)
            nc.sync.dma_start(out=vT[:, NT // 2:NT, :], in_=vT_r[:, NT // 2:NT, :])
            nc.sync.dma_start(out=vT[:, 0:NT // 2, :], in_=vT_r[:, 0:NT // 2, :])

            # bf16 x for the residual add (needed only in phase OUT)
            xsb = [per.tile([128, NP], BF16, tag=f"xb{ci}", name=f"xb{ci}")
                   for ci in range(CT)]
            for ci in range(CT):
                nc.sync.dma_start(out=xsb[ci], in_=xb_d[ci, :, :])

            # ---- persistent intermediates --------------------------------
            kT = per.tile([128, NT, 256], BF16, tag="kT", name="kT")
            qphi = [per.tile([128, N], BF16, tag=f"qphi{ct}", name=f"qphi{ct}")
                    for ct in range(CT)]
            kv_sb = per.tile([128, CT, 256], BF16, tag="kv", name="kv_sb")

            # ---- phase NT: k^T (conv in transposed layout) ---------------
            # Two adjacent n-tiles share one PSUM bank and one phi chain.
            # The kv matmuls ride along incrementally (PE has slack here),
            # so no serial KV phase separates Q from OUT.
            kv_ps = [ps2.tile([128, 256], F32, tag="kvp", name=f"kv_ps{ch}")
                     for ch in range(CT)]
            # PSUM holds raw = 64*(conv + bk + 1); the rank-1 bf16 start
            # matmul ones^T @ [64(bk+1)|64(bk+1)] seeds bias+1, fp8 conv
            # matmuls accumulate on top.
            for ip in range(NT // 2):
                kt_ps = ps.tile([128, 512], F32, tag="bank", name="kt_ps")
                nc.tensor.matmul(kt_ps, ones128, bk2, start=True, stop=False)
                for h in range(2):
                    off = (ip * 2 + h) * 128
                    half = kt_ps[:, h * 256:(h + 1) * 256]
                    for t, (src_t, o) in enumerate(
                            ((x8, 0), (x8s, 0), (x8, 2))):
                        nc.tensor.matmul(
                            half,
                            src_t[:, :, off + o:off + o + 128],
                            w8k[:, t, :, :],
                            start=False,
                            stop=(t == 2),
                            perf_mode=mybir.MatmulPerfMode.DoubleRow,
                        )
                # 64*phi = max(raw, min(64*exp(z-1), 64)): one ACT exp
                # straight from PSUM, one DVE stt. kT carries 64*phi.
                e = tmp.tile([128, 512], F32, tag="nte", name="e_nt")
                nc.scalar.activation(
                    e, kt_ps, AF.Exp, scale=1.0 / 64.0, bias=bconst[:, 0:1])
                nc.vector.scalar_tensor_tensor(
                    kT[:, ip * 2:ip * 2 + 2, :].rearrange("p i d -> p (i d)"),
                    e, 64.0, kt_ps, ALU.min, ALU.max)
                # kv matmuls for the FIRST 16 tiles are deferred into
                # Q's PE slack: the in-order PE stream would otherwise
                # block on vT's 2MB transfer during early NT pairs.
                if ip >= NT // 4:
                    for ch in range(CT):
                        for ii in (ip * 2, ip * 2 + 1):
                            nc.tensor.matmul(
                                kv_ps[ch],
                                kT[:, ii, ch * 128:(ch + 1) * 128],
                                vT[:, ii, :],
                                start=(ii == NT // 2),
                                stop=False,
                            )

            # ---- phase Q: conv q in [c, n] layout ------------------------
            # Same scheme; bias+1 per q-channel rides a rank-1 start matmul
            # (64(bq+1) as stationary row, ones as the moving operand).
            for ct in range(CT):
                bq64 = obk[0:1, 1024 + ct * 128:1024 + (ct + 1) * 128]
                for j in range(NJ):
                    q_ps = ps.tile([128, 512], F32, tag="bank", name="q_ps")
                    nc.tensor.matmul(q_ps, bq64, ones512, start=True, stop=False)
                    for t, (src_t, o) in enumerate(
                            ((x8, 0), (x8s, 0), (x8, 2))):
                        nc.tensor.matmul(
                            q_ps,
                            w8q[:, t, :, ct * 128:(ct + 1) * 128],
                            src_t[:, :, j * 512 + o:j * 512 + o + 512],
                            start=False,
                            stop=(t == 2),
                            perf_mode=mybir.MatmulPerfMode.DoubleRow,
                        )
                    e = tmp.tile([128, 512], F32, tag="qte", name="e_q")
                    nc.scalar.activation(
                        e, q_ps, AF.Exp, scale=1.0 / 64.0, bias=bconst[:, 0:1])
                    nc.vector.scalar_tensor_tensor(
                        qphi[ct][:, j * 512:(j + 1) * 512],
                        e, 64.0, q_ps, ALU.min, ALU.max)
                    if ct == 0:
                        # deferred kv tiles 2j, 2j+1 ride Q's PE slack
                        for ch in range(CT):
                            for ii in (j * 2, j * 2 + 1):
                                nc.tensor.matmul(
                                    kv_ps[ch],
                                    kT[:, ii, ch * 128:(ch + 1) * 128],
                                    vT[:, ii, :],
                                    start=False,
                                    stop=(ii == NT // 2 - 1),
                                )
                        if j == NJ - 1:
                            for ch in range(CT):
                                nc.vector.tensor_copy(
                                    kv_sb[:, ch, :], kv_ps[ch])

            # ---- phase OUT: out[d, n] = gelu(sum_c kv[c, d] q[c, n]) + x -
            # Stores batched in 1024-col bf16 pairs.
            for dt in range(CT):
                ob = None
                for j in range(NJ):
                    o_ps = ps.tile([128, 512], F32, tag="bank", name="o_ps")
                    for ch in range(CT):
                        nc.tensor.matmul(
                            o_ps,
                            kv_sb[:, ch, dt * 128:(dt + 1) * 128],
                            qphi[ch][:, j * 512:(j + 1) * 512],
                            start=(ch == 0),
                            stop=(ch == CT - 1),
                        )
                    g = tmp.tile([128, 512], BF16, tag="og", name="g_out")
                    nc.scalar.activation(g, o_ps, AF.Gelu, scale=1.0 / 4096.0)
                    if j % 2 == 0:
                        ob = obuf.tile([128, 1024], BF16, tag="ob", name="ob")
                    nc.vector.tensor_tensor(
                        ob[:, (j % 2) * 512:(j % 2) * 512 + 512],
                        g, xsb[dt][:, j * 512 + 1:j * 512 + 513], ALU.add)
                    if j % 2 == 1:
                        nc.sync.dma_start(
                            out=out_d[dt * 128:(dt + 1) * 128,
                                      (j - 1) * 512:(j + 1) * 512],
                            in_=ob,
                        )

    nc.compile()
    return nc


_NC_CACHE = None


def _get_nc():
    global _NC_CACHE
    if _NC_CACHE is None:
        _NC_CACHE = _build_nc()
    return _NC_CACHE


def _prep(x, conv_w, conv_b):
    x = np.asarray(x, dtype=np.float32)
    conv_w = np.asarray(conv_w, dtype=np.float32)
    conv_b = np.asarray(conv_b, dtype=np.float32)
    xb = np.zeros((B, CT, 128, NP), dtype=BF)
    xb[:, :, :, 1:N + 1] = x.reshape(B, CT, 128, N).astype(BF)
    # x8[b, p, ci, n] = padded x[b, ci*128+p, n] for DoubleRow rhs/lhsT;
    # row pitch NPP (16B-aligned); x8s = shifted one column left (t=1 tap)
    xpad = np.zeros((B, CT, 128, NPP), dtype=np.float32)
    xpad[:, :, :, 1:N + 1] = x.reshape(B, CT, 128, N)
    x8 = np.ascontiguousarray(
        xpad.transpose(0, 2, 1, 3)).reshape(B, 128, CT * NPP).astype(F8)
    xsh = np.zeros((B, CT, 128, NPP), dtype=np.float32)
    xsh[:, :, :, 0:NPP - 1] = xpad[:, :, :, 1:NPP]
    x8s = np.ascontiguousarray(
        xsh.transpose(0, 2, 1, 3)).reshape(B, 128, CT * NPP).astype(F8)
    # xt[b, p, i*256 + d] = x[b, d, i*128 + p]  (v^T tiled for contiguous DMA)
    xt = np.ascontiguousarray(
        x.transpose(0, 2, 1).reshape(B, NT, 128, C).transpose(0, 2, 1, 3)
    ).reshape(B, 128, NT * C).astype(BF)
    # w[t, ci_t, p, co] = conv_w[co, ci_t*128 + p, t]; slot = ci*3 + t
    w = conv_w.transpose(2, 1, 0).reshape(3, CT, 128, 2 * C)
    w = w.transpose(1, 0, 2, 3)                      # [ci, t, p, co]
    # slot order (t, ci) for DoubleRow: [p, t, ci, co]
    w8q = np.ascontiguousarray(
        w[:, :, :, :C].transpose(2, 1, 0, 3) * WS).reshape(128, 6 * 256).astype(F8)
    w8k = np.ascontiguousarray(
        w[:, :, :, C:].transpose(2, 1, 0, 3) * WS).reshape(128, 6 * 256).astype(F8)
    obk = np.ones((1, 1280), dtype=np.float32)
    obk[0, 512:768] = WS * (conv_b[C:] + 1.0)
    obk[0, 768:1024] = WS * (conv_b[C:] + 1.0)
    obk[0, 1024:1280] = WS * (conv_b[:C] + 1.0)
    obk = obk.astype(BF)
    return x8, x8s, xb, xt, w8k, w8q, obk


def make_in_maps(x, conv_w, conv_b):
    x8, x8s, xb, xt, w8k, w8q, obk = _prep(x, conv_w, conv_b)
    return [
        {"x8": x8[b], "x8s": x8s[b], "xb": xb[b], "xt": xt[b],
         "w8k": w8k, "w8q": w8q, "obk": obk}
        for b in range(B)
    ]


def kernel(x: np.ndarray, conv_w: np.ndarray, conv_b: np.ndarray) -> np.ndarray:
    nc = _get_nc()
    in_maps = make_in_maps(x, conv_w, conv_b)
    res = run_bass_kernel_spmd(nc, in_maps, core_ids=list(range(NCORES)))
    return np.stack([res.results[b]["out"] for b in range(B)],
                    axis=0).astype(np.float32)



# revision 3
# speedup vs baseline: 3.9572x; 3.9572x over previous
"""Trainium2 Bass kernel for nn_AttentionLayer (conv1d -> linear attention -> gelu + residual).

Full inputs:  x [8, 256, 4096] f32, conv_w [512, 256, 3] f32, conv_b [512] f32
Full output:  [8, 256, 4096] f32

Sharding: pure data-parallel over batch B=8 -> 8 NeuronCores, one batch each.

The graded metric is wall-clock per kernel() call, and the axon tunnel to the
device moves ~46 MB/s with ~80 ms fixed RPC latency per dispatch, while the
on-device math is ~0.15 ms.  So this kernel is organized entirely around wire
bytes:

  UP   (11.6 MB): x int8-quantized per [b, channel] row (scale = absmax/127)
                  + f32 scales + fp8(x64) conv weights + bias consts.
  DOWN ( 8.4 MB): g = gelu(attention) int8-quantized per [b, channel] row
                  (device computes per-row absmax) + f32 scales.
  Residual "+ x" happens on the HOST, where exact f32 x is free, so neither a
  bf16 x copy (residual) nor a bf16 g needs to cross the wire.  int8 rows
  measure ~1.0e-2 end-to-end rel err (gate 2e-2): x rows are Gaussian and g
  rows have absmax/rms ~ 6, so uniform per-row quantization stays ~1% rms.

Everything else the math needs is derived on device from the int8 x:
  xb   bf16 = int8 x * row scale       (ACT copy, per-partition scale operand)
  x8   fp8  = xb                       (DVE copy; conv rhs/lhsT, DoubleRow)
  x8s  fp8  = xb shifted one column    (t=1 conv tap; dual-fp8 needs even offsets)
  vT   bf16 = x^T, 64 PE 128x128 transposes via identity matmul
The conv matmuls (75% of FLOPs) run fp8 E4M3 at 2x PE rate with weights
host-scaled by 64 (else subnormal); the 1/64 and phi's "+1" fold into the phi
chain:  with raw = 64*(conv + bias + 1) in PSUM,
  64*phi = max(raw, min(64*exp(raw/64 + ln64 - 1), 64))
so ACT does one exp straight from PSUM and DVE one fused min/max.  kv and
q@(kv) stay bf16 (kv entries get no averaging benefit from fp8).

Per-core math (C=256, N=4096, one batch):
  y  = conv1d(x, w, pad=1) + b            # [2C, N]
  q  = phi(y[:C]), k = phi(y[C:])         # phi = elu+1
  kv = k^T @ x^T                          # [C, C]   (v = x)
  g  = gelu(q @ kv)                       # [C, N]   -> int8 rows + scales
  (host) out = g * scale + x

The runner dispatches one cached pjit (shard_map over 8 cores) per call --
rebuilding it per call (as bass_utils.run_bass_kernel_spmd does) re-traces and
re-dispatches ~0.2 s of XLA work, and its donated output buffers would upload
another 8 MB of host zeros; here the donated buffers are created device-side.
"""

import numpy as np
import ml_dtypes

import jax
import jax.numpy as jnp
from jax.sharding import Mesh, NamedSharding, PartitionSpec

import concourse.bass as bass
import concourse.mybir as mybir
import concourse.tile as tile
from concourse import bacc
from concourse.masks import make_identity

F32 = mybir.dt.float32
BF16 = mybir.dt.bfloat16
FP8 = mybir.dt.float8e4
I8 = mybir.dt.int8
AF = mybir.ActivationFunctionType
ALU = mybir.AluOpType

B, C, N = 8, 256, 4096
NCORES = 8
CT = C // 128         # 2 c-tiles (partition groups) per 256-channel dim
NJ = N // 512         # 8 column chunks of 512
NT = N // 128         # 32 n-tiles of 128
NP = N + 2            # x padded with one zero column on each side
NPP = 4112            # x8 row pitch: NP padded so the dual-fp8 outer stride
                      # stays 16B-aligned
WS = 64.0             # fp8 weight scale

BF = ml_dtypes.bfloat16
F8 = ml_dtypes.float8_e4m3


def _build_nc():
    nc = bacc.Bacc("TRN2", target_bir_lowering=False, debug=False, num_devices=NCORES)

    xi_d = nc.declare_dram_parameter("xi", [CT, 128, NP], I8, isOutput=False)
    scl_d = nc.declare_dram_parameter("scl", [128, CT], F32, isOutput=False)
    w8k_d = nc.declare_dram_parameter("w8k", [128, 6 * 256], FP8, isOutput=False)
    w8q_d = nc.declare_dram_parameter("w8q", [128, 6 * 256], FP8, isOutput=False)
    obk_d = nc.declare_dram_parameter("obk", [1, 1280], BF16, isOutput=False)
    og_d = nc.declare_dram_parameter("og", [C, N], I8, isOutput=True)
    os_d = nc.declare_dram_parameter("os", [128, CT], F32, isOutput=True)

    with tile.TileContext(nc) as tc:
        with (
            tc.tile_pool(name="persist", bufs=1) as per,
            tc.tile_pool(name="tmp", bufs=4) as tmp,
            tc.tile_pool(name="psum", bufs=4, space="PSUM") as ps,
            tc.tile_pool(name="psum2", bufs=2, space="PSUM") as ps2,
            tc.tile_pool(name="psumT", bufs=2, space="PSUM") as pst,
        ):
            # ---- inputs ------------------------------------------------
            obk = per.tile([1, 1280], BF16, tag="obk", name="obk")
            nc.sync.dma_start(out=obk, in_=obk_d[:, :])
            ones128 = obk[0:1, 0:128]
            ones512 = obk[0:1, 0:512]
            bk2 = obk[0:1, 512:1024]
            scl = per.tile([128, CT], F32, tag="scl", name="scl")
            nc.sync.dma_start(out=scl, in_=scl_d[:, :])
            xi = [per.tile([128, NP], I8, tag=f"xi{ci}", name=f"xi{ci}")
                  for ci in range(CT)]
            for ci in range(CT):
                nc.sync.dma_start(out=xi[ci], in_=xi_d[ci, :, :])
            w8k = per.tile([128, 3, 2, 256], FP8, tag="w8k", name="w8k")
            nc.sync.dma_start(
                out=w8k, in_=w8k_d.rearrange("p (t i c) -> p t i c", i=2, c=256))
            w8q = per.tile([128, 3, 2, 256], FP8, tag="w8q", name="w8q")
            nc.sync.dma_start(
                out=w8q, in_=w8q_d.rearrange("p (t i c) -> p t i c", i=2, c=256))

            # Warm the ACT Exp table early (must read initialized SBUF).
            warm = tmp.tile([1, 1], F32, tag="warm", name="warm")
            nc.scalar.activation(warm, obk[0:1, 0:1], AF.Exp)
            # exp bias const ln(64)-1 (phi stored x64, clamp moved after exp)
            bconst = per.tile([128, 1], F32, tag="bconst", name="bconst")
            nc.gpsimd.memset(bconst, 3.1588830833596715)
            ident = per.tile([128, 128], BF16, tag="ident", name="ident")
            make_identity(nc, ident)

            # ---- derive xb (bf16), x8/x8s (fp8), vT (x^T bf16) ---------
            xb = [per.tile([128, NP], BF16, tag=f"xb{ci}", name=f"xb{ci}")
                  for ci in range(CT)]
            for ci in range(CT):
                nc.scalar.activation(xb[ci], xi[ci], AF.Copy,
                                     scale=scl[:, ci:ci + 1])
            x8 = per.tile([128, CT, NPP], FP8, tag="x8", name="x8")
            x8s = per.tile([128, CT, NPP], FP8, tag="x8s", name="x8s")
            for ci in range(CT):
                # tail cols [NP:NPP) are never read by the conv taps
                nc.vector.tensor_copy(x8[:, ci, 0:NP], xb[ci])
                nc.vector.tensor_copy(x8s[:, ci, 0:NP - 1], xb[ci][:, 1:NP])

            vT = per.tile([128, NT, 256], BF16, tag="vT", name="vT")
            for i in range(NT):
                for ci in range(CT):
                    psT = pst.tile([128, 128], BF16, tag="psT", name="psT")
                    nc.tensor.transpose(
                        psT, xb[ci][:, 1 + i * 128:1 + (i + 1) * 128], ident)
                    nc.vector.tensor_copy(vT[:, i, ci * 128:(ci + 1) * 128], psT)

            # ---- persistent intermediates ------------------------------
            kT = per.tile([128, NT, 256], BF16, tag="kT", name="kT")
            qphi = [per.tile([128, N], BF16, tag=f"qphi{ct}", name=f"qphi{ct}")
                    for ct in range(CT)]
            kv_sb = per.tile([128, CT, 256], BF16, tag="kv", name="kv_sb")

            # ---- phase NT: k^T conv (transposed layout) + fused kv -----
            # Two adjacent n-tiles share one PSUM bank and one phi chain.
            kv_ps = [ps2.tile([128, 256], F32, tag="kvp", name=f"kv_ps{ch}")
                     for ch in range(CT)]
            for ip in range(NT // 2):
                kt_ps = ps.tile([128, 512], F32, tag="bank", name="kt_ps")
                nc.tensor.matmul(kt_ps, ones128, bk2, start=True, stop=False)
                for h in range(2):
                    off = (ip * 2 + h) * 128
                    half = kt_ps[:, h * 256:(h + 1) * 256]
                    for t, (src_t, o) in enumerate(
                            ((x8, 0), (x8s, 0), (x8, 2))):
                        nc.tensor.matmul(
                            half,
                            src_t[:, :, off + o:off + o + 128],
                            w8k[:, t, :, :],
                            start=False,
                            stop=(t == 2),
                            perf_mode=mybir.MatmulPerfMode.DoubleRow,
                        )
                # 64*phi = max(raw, min(64*exp(raw/64 + ln64 - 1), 64))
                e = tmp.tile([128, 512], F32, tag="nte", name="e_nt")
                nc.scalar.activation(
                    e, kt_ps, AF.Exp, scale=1.0 / 64.0, bias=bconst[:, 0:1])
                nc.vector.scalar_tensor_tensor(
                    kT[:, ip * 2:ip * 2 + 2, :].rearrange("p i d -> p (i d)"),
                    e, 64.0, kt_ps, ALU.min, ALU.max)
                for ch in range(CT):
                    for ii in (ip * 2, ip * 2 + 1):
                        nc.tensor.matmul(
                            kv_ps[ch],
                            kT[:, ii, ch * 128:(ch + 1) * 128],
                            vT[:, ii, :],
                            start=(ii == 0),
                            stop=(ii == NT - 1),
                        )
            for ch in range(CT):
                nc.vector.tensor_copy(kv_sb[:, ch, :], kv_ps[ch])

            # ---- phase Q: conv q in [c, n] layout ----------------------
            for ct in range(CT):
                bq64 = obk[0:1, 1024 + ct * 128:1024 + (ct + 1) * 128]
                for j in range(NJ):
                    q_ps = ps.tile([128, 512], F32, tag="bank", name="q_ps")
                    nc.tensor.matmul(q_ps, bq64, ones512, start=True, stop=False)
                    for t, (src_t, o) in enumerate(
                            ((x8, 0), (x8s, 0), (x8, 2))):
                        nc.tensor.matmul(
                            q_ps,
                            w8q[:, t, :, ct * 128:(ct + 1) * 128],
                            src_t[:, :, j * 512 + o:j * 512 + o + 512],
                            start=False,
                            stop=(t == 2),
                            perf_mode=mybir.MatmulPerfMode.DoubleRow,
                        )
                    e = tmp.tile([128, 512], F32, tag="qte", name="e_q")
                    nc.scalar.activation(
                        e, q_ps, AF.Exp, scale=1.0 / 64.0, bias=bconst[:, 0:1])
                    nc.vector.scalar_tensor_tensor(
                        qphi[ct][:, j * 512:(j + 1) * 512],
                        e, 64.0, q_ps, ALU.min, ALU.max)

            # ---- phase OUT: g = gelu(q@kv), int8 rows + scales ---------
            os_sb = per.tile([128, CT], F32, tag="os", name="os_sb")
            for dt in range(CT):
                gb = per.tile([128, N], BF16, tag=f"gb{dt}", name=f"gb{dt}")
                for j in range(NJ):
                    o_ps = ps.tile([128, 512], F32, tag="bank", name="o_ps")
                    for ch in range(CT):
                        nc.tensor.matmul(
                            o_ps,
                            kv_sb[:, ch, dt * 128:(dt + 1) * 128],
                            qphi[ch][:, j * 512:(j + 1) * 512],
                            start=(ch == 0),
                            stop=(ch == CT - 1),
                        )
                    nc.scalar.activation(gb[:, j * 512:(j + 1) * 512], o_ps,
                                         AF.Gelu, scale=1.0 / 4096.0)
                gm = tmp.tile([128, 1], F32, tag="gm", name="gm")
                nc.vector.tensor_reduce(gm, gb, mybir.AxisListType.X, ALU.max,
                                        apply_absolute_value=True)
                nc.vector.tensor_scalar(gm, gm, 1e-30, None, ALU.max)
                inv = tmp.tile([128, 1], F32, tag="inv", name="inv")
                nc.vector.reciprocal(inv, gm)
                og = per.tile([128, N], I8, tag=f"og{dt}", name=f"og{dt}")
                nc.vector.tensor_scalar(og, gb, inv[:, 0:1], 127.0,
                                        ALU.mult, ALU.mult)
                nc.vector.tensor_scalar(os_sb[:, dt:dt + 1], gm, 1.0 / 127.0,
                                        None, ALU.mult)
                nc.sync.dma_start(out=og_d[dt * 128:(dt + 1) * 128, :], in_=og)
            nc.sync.dma_start(out=os_d[:, :], in_=os_sb)

    nc.compile()
    return nc


def _prep(x, conv_w, conv_b):
    x = np.asarray(x, dtype=np.float32)
    conv_w = np.asarray(conv_w, dtype=np.float32)
    conv_b = np.asarray(conv_b, dtype=np.float32)

    # int8 per-[b, channel] row quantization of x (absmax -> +-127)
    rs = np.abs(x).max(axis=2)
    np.maximum(rs, 1e-30, out=rs)
    rs /= 127.0
    t = x / rs[:, :, None]
    np.rint(t, out=t)
    xi = np.zeros((B, CT, 128, NP), dtype=np.int8)
    xi[:, :, :, 1:N + 1] = t.astype(np.int8).reshape(B, CT, 128, N)
    scl = np.ascontiguousarray(
        rs.reshape(B, CT, 128).transpose(0, 2, 1)).astype(np.float32)

    # conv weights x64 in fp8, DoubleRow slot layout [p, t, ci, co]
    w = conv_w.transpose(2, 1, 0).reshape(3, CT, 128, 2 * C)
    w = w.transpose(1, 0, 2, 3)                      # [ci, t, p, co]
    w8q = np.ascontiguousarray(
        w[:, :, :, :C].transpose(2, 1, 0, 3) * WS).reshape(128, 6 * 256).astype(F8)
    w8k = np.ascontiguousarray(
        w[:, :, :, C:].transpose(2, 1, 0, 3) * WS).reshape(128, 6 * 256).astype(F8)
    obk = np.ones((1, 1280), dtype=np.float32)
    obk[0, 512:768] = WS * (conv_b[C:] + 1.0)
    obk[0, 768:1024] = WS * (conv_b[C:] + 1.0)
    obk[0, 1024:1280] = WS * (conv_b[:C] + 1.0)
    obk = obk.astype(BF)
    return xi, scl, w8k, w8q, obk


_STATE = None


def _get_state():
    global _STATE
    if _STATE is None:
        from concourse.bass2jax import (
            _bass_exec_p, install_neuronx_cc_hook, partition_id_tensor)
        from jax.experimental.shard_map import shard_map

        nc = _build_nc()
        install_neuronx_cc_hook()

        partition_name = (nc.partition_id_tensor.name
                          if nc.partition_id_tensor else None)
        in_names, out_names, out_avals = [], [], []
        for alloc in nc.m.functions[0].allocations:
            if not isinstance(alloc, mybir.MemoryLocationSet):
                continue
            name = alloc.memorylocations[0].name
            if alloc.kind == "ExternalInput":
                if name != partition_name:
                    in_names.append(name)
            elif alloc.kind == "ExternalOutput":
                out_names.append(name)
                out_avals.append(jax.core.ShapedArray(
                    tuple(alloc.tensor_shape), mybir.dt.np(alloc.dtype)))
        dbg_zero = {}
        if nc.dbg_addr is not None:
            dbg_zero = {nc.dbg_addr.name: np.zeros((1, 2), np.uint32)}
            if nc.dbg_addr.name not in in_names:
                in_names.append(nc.dbg_addr.name)
        n_params = len(in_names)
        n_outs = len(out_names)
        all_names = in_names + out_names
        if partition_name is not None:
            all_names.append(partition_name)

        def _body(*args):
            operands = list(args)
            if partition_name is not None:
                operands.append(partition_id_tensor())
            return tuple(_bass_exec_p.bind(
                *operands,
                out_avals=tuple(out_avals),
                in_names=tuple(all_names),
                out_names=tuple(out_names),
                lowering_input_output_aliases=(),
                sim_require_finite=True,
                sim_require_nnan=True,
                nc=nc,
            ))

        devices = jax.devices()[:NCORES]
        mesh = Mesh(np.asarray(devices), ("core",))
        sharded = jax.jit(
            shard_map(_body, mesh=mesh,
                      in_specs=(PartitionSpec("core"),) * (n_params + n_outs),
                      out_specs=(PartitionSpec("core"),) * n_outs,
                      check_rep=False),
            donate_argnums=tuple(range(n_params, n_params + n_outs)),
            keep_unused=True,
        )
        # Donated output buffers built on device (uploading host zeros would
        # cost another ~8 MB of wire per call).
        zero_shapes = [(NCORES * a.shape[0], *a.shape[1:]) for a in out_avals]
        zero_dtypes = [a.dtype for a in out_avals]
        sh = NamedSharding(mesh, PartitionSpec("core"))
        zeros_fn = jax.jit(
            lambda: tuple(jnp.zeros(s, d)
                          for s, d in zip(zero_shapes, zero_dtypes)),
            out_shardings=(sh,) * n_outs,
        )
        _STATE = {
            "in_names": in_names,
            "out_names": out_names,
            "sharded": sharded,
            "zeros_fn": zeros_fn,
            "dbg_zero": dbg_zero,
        }
    return _STATE


def kernel(x: np.ndarray, conv_w: np.ndarray, conv_b: np.ndarray) -> np.ndarray:
    st = _get_state()
    xi, scl, w8k, w8q, obk = _prep(x, conv_w, conv_b)
    params = {
        "xi": xi.reshape(B * CT, 128, NP),
        "scl": scl.reshape(B * 128, CT),
        "w8k": np.broadcast_to(w8k, (B, 128, 6 * 256)).reshape(B * 128, 6 * 256),
        "w8q": np.broadcast_to(w8q, (B, 128, 6 * 256)).reshape(B * 128, 6 * 256),
        "obk": np.broadcast_to(obk, (B, 1, 1280)).reshape(B, 1280),
    }
    for name, z in st["dbg_zero"].items():
        params[name] = np.broadcast_to(z, (B * z.shape[0], z.shape[1]))
    zeros = st["zeros_fn"]()
    outs = st["sharded"](*[np.ascontiguousarray(params[n])
                           for n in st["in_names"]], *zeros)
    out_map = dict(zip(st["out_names"], outs))
    og = np.asarray(out_map["og"]).reshape(B, C, N)
    osc = np.asarray(out_map["os"]).reshape(B, 128, CT).transpose(0, 2, 1)
    out = og * osc.reshape(B, C, 1).astype(np.float32)
    out += np.asarray(x, np.float32)
    return out


# revision 6
# speedup vs baseline: 4.5220x; 1.1427x over previous
"""Trainium2 Bass kernel for nn_AttentionLayer (conv1d -> linear attention -> gelu + residual).

Full inputs:  x [8, 256, 4096] f32, conv_w [512, 256, 3] f32, conv_b [512] f32
Full output:  [8, 256, 4096] f32

Sharding: pure data-parallel over batch B=8 -> 8 NeuronCores, one batch each.

The graded metric is wall-clock per kernel() call, and the axon tunnel to the
device moves ~46 MB/s with ~80 ms fixed RPC latency per dispatch, while the
on-device math is ~0.15 ms.  So this kernel is organized entirely around wire
bytes:

  UP   (11.6 MB): x int8-quantized per [b, channel] row (scale = absmax/127)
                  + f32 scales + fp8(x64) conv weights + bias consts.
  DOWN ( 8.4 MB): g = gelu(attention) int8-quantized per [b, channel] row
                  (device computes per-row absmax) + f32 scales.
  Residual "+ x" happens on the HOST, where exact f32 x is free, so neither a
  bf16 x copy (residual) nor a bf16 g needs to cross the wire.  int8 rows
  measure ~1.0e-2 end-to-end rel err (gate 2e-2): x rows are Gaussian and g
  rows have absmax/rms ~ 6, so uniform per-row quantization stays ~1% rms.

Everything else the math needs is derived on device from the int8 x:
  xb   bf16 = int8 x * row scale       (ACT copy, per-partition scale operand)
  x8   fp8  = xb                       (DVE copy; conv rhs/lhsT, DoubleRow)
  x8s  fp8  = xb shifted one column    (t=1 conv tap; dual-fp8 needs even offsets)
  vT   bf16 = x^T, 64 PE 128x128 transposes via identity matmul
The conv matmuls (75% of FLOPs) run fp8 E4M3 at 2x PE rate with weights
host-scaled by 64 (else subnormal); the 1/64 and phi's "+1" fold into the phi
chain:  with raw = 64*(conv + bias + 1) in PSUM,
  64*phi = max(raw, min(64*exp(raw/64 + ln64 - 1), 64))
so ACT does one exp straight from PSUM and DVE one fused min/max.  kv and
q@(kv) stay bf16 (kv entries get no averaging benefit from fp8).

Per-core math (C=256, N=4096, one batch):
  y  = conv1d(x, w, pad=1) + b            # [2C, N]
  q  = phi(y[:C]), k = phi(y[C:])         # phi = elu+1
  kv = k^T @ x^T                          # [C, C]   (v = x)
  g  = gelu(q @ kv)                       # [C, N]   -> int8 rows + scales
  (host) out = g * scale + x

The runner dispatches one cached pjit (shard_map over 8 cores) per call --
rebuilding it per call (as bass_utils.run_bass_kernel_spmd does) re-traces and
re-dispatches ~0.2 s of XLA work, and its donated output buffers would upload
another 8 MB of host zeros; here the donated buffers are created device-side.
"""

import numpy as np
import ml_dtypes

import jax
import jax.numpy as jnp
from jax.sharding import Mesh, NamedSharding, PartitionSpec

import concourse.bass as bass
import concourse.mybir as mybir
import concourse.tile as tile
from concourse import bacc
from concourse.masks import make_identity

F32 = mybir.dt.float32
BF16 = mybir.dt.bfloat16
FP8 = mybir.dt.float8e4
I8 = mybir.dt.int8
AF = mybir.ActivationFunctionType
ALU = mybir.AluOpType

B, C, N = 8, 256, 4096
NCORES = 8
CT = C // 128         # 2 c-tiles (partition groups) per 256-channel dim
NJ = N // 512         # 8 column chunks of 512
NT = N // 128         # 32 n-tiles of 128
NP = N + 2            # x padded with one zero column on each side
NPP = 4112            # x8 row pitch: NP padded so the dual-fp8 outer stride
                      # stays 16B-aligned
WS = 64.0             # fp8 weight scale

BF = ml_dtypes.bfloat16
F8 = ml_dtypes.float8_e4m3


def _build_nc():
    nc = bacc.Bacc("TRN2", target_bir_lowering=False, debug=False, num_devices=NCORES)

    xi_d = nc.declare_dram_parameter("xi", [CT, 128, NP], I8, isOutput=False)
    scl_d = nc.declare_dram_parameter("scl", [128, CT], F32, isOutput=False)
    w8k_d = nc.declare_dram_parameter("w8k", [128, 6 * 256], FP8, isOutput=False)
    w8q_d = nc.declare_dram_parameter("w8q", [128, 6 * 256], FP8, isOutput=False)
    obk_d = nc.declare_dram_parameter("obk", [1, 1280], BF16, isOutput=False)
    og_d = nc.declare_dram_parameter("og", [C, N], I8, isOutput=True)
    os_d = nc.declare_dram_parameter("os", [128, CT], F32, isOutput=True)

    with tile.TileContext(nc) as tc:
        with (
            tc.tile_pool(name="persist", bufs=1) as per,
            tc.tile_pool(name="tmp", bufs=4) as tmp,
            tc.tile_pool(name="psum", bufs=4, space="PSUM") as ps,
            tc.tile_pool(name="psum2", bufs=2, space="PSUM") as ps2,
            tc.tile_pool(name="psumT", bufs=2, space="PSUM") as pst,
        ):
            # ---- inputs ------------------------------------------------
            obk = per.tile([1, 1280], BF16, tag="obk", name="obk")
            nc.sync.dma_start(out=obk, in_=obk_d[:, :])
            ones128 = obk[0:1, 0:128]
            ones512 = obk[0:1, 0:512]
            bk2 = obk[0:1, 512:1024]
            scl = per.tile([128, CT], F32, tag="scl", name="scl")
            nc.sync.dma_start(out=scl, in_=scl_d[:, :])
            xi = [per.tile([128, NP], I8, tag=f"xi{ci}", name=f"xi{ci}")
                  for ci in range(CT)]
            for ci in range(CT):
                nc.sync.dma_start(out=xi[ci], in_=xi_d[ci, :, :])
            w8k = per.tile([128, 3, 2, 256], FP8, tag="w8k", name="w8k")
            nc.sync.dma_start(
                out=w8k, in_=w8k_d.rearrange("p (t i c) -> p t i c", i=2, c=256))
            w8q = per.tile([128, 3, 2, 256], FP8, tag="w8q", name="w8q")
            nc.sync.dma_start(
                out=w8q, in_=w8q_d.rearrange("p (t i c) -> p t i c", i=2, c=256))

            # Warm the ACT Exp table early (must read initialized SBUF).
            warm = tmp.tile([1, 1], F32, tag="warm", name="warm")
            nc.scalar.activation(warm, obk[0:1, 0:1], AF.Exp)
            # exp bias const ln(64)-1 (phi stored x64, clamp moved after exp)
            bconst = per.tile([128, 1], F32, tag="bconst", name="bconst")
            nc.gpsimd.memset(bconst, 3.1588830833596715)
            ident = per.tile([128, 128], BF16, tag="ident", name="ident")
            make_identity(nc, ident)

            # ---- derive xb (bf16), x8/x8s (fp8), vT (x^T bf16) ---------
            xb = [per.tile([128, NP], BF16, tag=f"xb{ci}", name=f"xb{ci}")
                  for ci in range(CT)]
            for ci in range(CT):
                nc.scalar.activation(xb[ci], xi[ci], AF.Copy,
                                     scale=scl[:, ci:ci + 1])
            x8 = per.tile([128, CT, NPP], FP8, tag="x8", name="x8")
            x8s = per.tile([128, CT, NPP], FP8, tag="x8s", name="x8s")
            for ci in range(CT):
                # tail cols [NP:NPP) are never read by the conv taps
                nc.vector.tensor_copy(x8[:, ci, 0:NP], xb[ci])
                nc.vector.tensor_copy(x8s[:, ci, 0:NP - 1], xb[ci][:, 1:NP])

            vT = per.tile([128, NT, 256], BF16, tag="vT", name="vT")
            for i in range(NT):
                for ci in range(CT):
                    psT = pst.tile([128, 128], BF16, tag="psT", name="psT")
                    nc.tensor.transpose(
                        psT, xb[ci][:, 1 + i * 128:1 + (i + 1) * 128], ident)
                    nc.vector.tensor_copy(vT[:, i, ci * 128:(ci + 1) * 128], psT)

            # ---- persistent intermediates ------------------------------
            kT = per.tile([128, NT, 256], BF16, tag="kT", name="kT")
            qphi = [per.tile([128, N], BF16, tag=f"qphi{ct}", name=f"qphi{ct}")
                    for ct in range(CT)]
            kv_sb = per.tile([128, CT, 256], BF16, tag="kv", name="kv_sb")

            # ---- phase NT: k^T conv (transposed layout) + fused kv -----
            # Two adjacent n-tiles share one PSUM bank and one phi chain.
            kv_ps = [ps2.tile([128, 256], F32, tag="kvp", name=f"kv_ps{ch}")
                     for ch in range(CT)]
            for ip in range(NT // 2):
                kt_ps = ps.tile([128, 512], F32, tag="bank", name="kt_ps")
                nc.tensor.matmul(kt_ps, ones128, bk2, start=True, stop=False)
                for h in range(2):
                    off = (ip * 2 + h) * 128
                    half = kt_ps[:, h * 256:(h + 1) * 256]
                    for t, (src_t, o) in enumerate(
                            ((x8, 0), (x8s, 0), (x8, 2))):
                        nc.tensor.matmul(
                            half,
                            src_t[:, :, off + o:off + o + 128],
                            w8k[:, t, :, :],
                            start=False,
                            stop=(t == 2),
                            perf_mode=mybir.MatmulPerfMode.DoubleRow,
                        )
                # 64*phi = max(raw, min(64*exp(raw/64 + ln64 - 1), 64))
                e = tmp.tile([128, 512], F32, tag="nte", name="e_nt")
                nc.scalar.activation(
                    e, kt_ps, AF.Exp, scale=1.0 / 64.0, bias=bconst[:, 0:1])
                nc.vector.scalar_tensor_tensor(
                    kT[:, ip * 2:ip * 2 + 2, :].rearrange("p i d -> p (i d)"),
                    e, 64.0, kt_ps, ALU.min, ALU.max)
                for ch in range(CT):
                    for ii in (ip * 2, ip * 2 + 1):
                        nc.tensor.matmul(
                            kv_ps[ch],
                            kT[:, ii, ch * 128:(ch + 1) * 128],
                            vT[:, ii, :],
                            start=(ii == 0),
                            stop=(ii == NT - 1),
                        )
            for ch in range(CT):
                nc.vector.tensor_copy(kv_sb[:, ch, :], kv_ps[ch])

            # ---- phase Q: conv q in [c, n] layout ----------------------
            for ct in range(CT):
                bq64 = obk[0:1, 1024 + ct * 128:1024 + (ct + 1) * 128]
                for j in range(NJ):
                    q_ps = ps.tile([128, 512], F32, tag="bank", name="q_ps")
                    nc.tensor.matmul(q_ps, bq64, ones512, start=True, stop=False)
                    for t, (src_t, o) in enumerate(
                            ((x8, 0), (x8s, 0), (x8, 2))):
                        nc.tensor.matmul(
                            q_ps,
                            w8q[:, t, :, ct * 128:(ct + 1) * 128],
                            src_t[:, :, j * 512 + o:j * 512 + o + 512],
                            start=False,
                            stop=(t == 2),
                            perf_mode=mybir.MatmulPerfMode.DoubleRow,
                        )
                    e = tmp.tile([128, 512], F32, tag="qte", name="e_q")
                    nc.scalar.activation(
                        e, q_ps, AF.Exp, scale=1.0 / 64.0, bias=bconst[:, 0:1])
                    nc.vector.scalar_tensor_tensor(
                        qphi[ct][:, j * 512:(j + 1) * 512],
                        e, 64.0, q_ps, ALU.min, ALU.max)

            # ---- phase OUT: g = gelu(q@kv), int8 rows + scales ---------
            os_sb = per.tile([128, CT], F32, tag="os", name="os_sb")
            for dt in range(CT):
                gb = per.tile([128, N], BF16, tag=f"gb{dt}", name=f"gb{dt}")
                for j in range(NJ):
                    o_ps = ps.tile([128, 512], F32, tag="bank", name="o_ps")
                    for ch in range(CT):
                        nc.tensor.matmul(
                            o_ps,
                            kv_sb[:, ch, dt * 128:(dt + 1) * 128],
                            qphi[ch][:, j * 512:(j + 1) * 512],
                            start=(ch == 0),
                            stop=(ch == CT - 1),
                        )
                    nc.scalar.activation(gb[:, j * 512:(j + 1) * 512], o_ps,
                                         AF.Gelu, scale=1.0 / 4096.0)
                gm = tmp.tile([128, 1], F32, tag="gm", name="gm")
                nc.vector.tensor_reduce(gm, gb, mybir.AxisListType.X, ALU.max,
                                        apply_absolute_value=True)
                nc.vector.tensor_scalar(gm, gm, 1e-30, None, ALU.max)
                inv = tmp.tile([128, 1], F32, tag="inv", name="inv")
                nc.vector.reciprocal(inv, gm)
                og = per.tile([128, N], I8, tag=f"og{dt}", name=f"og{dt}")
                nc.vector.tensor_scalar(og, gb, inv[:, 0:1], 127.0,
                                        ALU.mult, ALU.mult)
                nc.vector.tensor_scalar(os_sb[:, dt:dt + 1], gm, 1.0 / 127.0,
                                        None, ALU.mult)
                nc.sync.dma_start(out=og_d[dt * 128:(dt + 1) * 128, :], in_=og)
            nc.sync.dma_start(out=os_d[:, :], in_=os_sb)

    nc.compile()
    return nc


def _prep(x, conv_w, conv_b):
    x = np.asarray(x, dtype=np.float32)
    conv_w = np.asarray(conv_w, dtype=np.float32)
    conv_b = np.asarray(conv_b, dtype=np.float32)

    # int8 per-[b, channel] row quantization of x (absmax -> +-127)
    rs = np.abs(x).max(axis=2)
    np.maximum(rs, 1e-30, out=rs)
    t = np.multiply(x, (127.0 / rs)[:, :, None])
    np.rint(t, out=t)
    global _XI_BUF
    if _XI_BUF is None:
        _XI_BUF = np.zeros((B, CT, 128, NP), dtype=np.int8)
    xi = _XI_BUF
    xi[:, :, :, 1:N + 1] = t.astype(np.int8).reshape(B, CT, 128, N)
    scl = np.ascontiguousarray(
        (rs * (1.0 / 127.0)).reshape(B, CT, 128).transpose(0, 2, 1))

    # conv weights x64 in fp8, DoubleRow slot layout [p, t, ci, co]
    w = conv_w.transpose(2, 1, 0).reshape(3, CT, 128, 2 * C)
    w = w.transpose(1, 0, 2, 3)                      # [ci, t, p, co]
    w8q = np.ascontiguousarray(
        w[:, :, :, :C].transpose(2, 1, 0, 3) * WS).reshape(128, 6 * 256).astype(F8)
    w8k = np.ascontiguousarray(
        w[:, :, :, C:].transpose(2, 1, 0, 3) * WS).reshape(128, 6 * 256).astype(F8)
    obk = np.ones((1, 1280), dtype=np.float32)
    obk[0, 512:768] = WS * (conv_b[C:] + 1.0)
    obk[0, 768:1024] = WS * (conv_b[C:] + 1.0)
    obk[0, 1024:1280] = WS * (conv_b[:C] + 1.0)
    obk = obk.astype(BF)
    return xi, scl, w8k, w8q, obk


_STATE = None
_XI_BUF = None


def _get_state():
    global _STATE
    if _STATE is None:
        from concourse.bass2jax import (
            _bass_exec_p, install_neuronx_cc_hook, partition_id_tensor)
        from jax.experimental.shard_map import shard_map

        nc = _build_nc()
        install_neuronx_cc_hook()

        partition_name = (nc.partition_id_tensor.name
                          if nc.partition_id_tensor else None)
        in_names, out_names, out_avals = [], [], []
        for alloc in nc.m.functions[0].allocations:
            if not isinstance(alloc, mybir.MemoryLocationSet):
                continue
            name = alloc.memorylocations[0].name
            if alloc.kind == "ExternalInput":
                if name != partition_name:
                    in_names.append(name)
            elif alloc.kind == "ExternalOutput":
                out_names.append(name)
                out_avals.append(jax.core.ShapedArray(
                    tuple(alloc.tensor_shape), mybir.dt.np(alloc.dtype)))
        dbg_zero = {}
        if nc.dbg_addr is not None:
            dbg_zero = {nc.dbg_addr.name: np.zeros((1, 2), np.uint32)}
            if nc.dbg_addr.name not in in_names:
                in_names.append(nc.dbg_addr.name)
        n_params = len(in_names)
        n_outs = len(out_names)
        all_names = in_names + out_names
        if partition_name is not None:
            all_names.append(partition_name)

        def _body(*args):
            operands = list(args)
            if partition_name is not None:
                operands.append(partition_id_tensor())
            return tuple(_bass_exec_p.bind(
                *operands,
                out_avals=tuple(out_avals),
                in_names=tuple(all_names),
                out_names=tuple(out_names),
                lowering_input_output_aliases=(),
                sim_require_finite=True,
                sim_require_nnan=True,
                nc=nc,
            ))

        devices = jax.devices()[:NCORES]
        mesh = Mesh(np.asarray(devices), ("core",))
        sharded = jax.jit(
            shard_map(_body, mesh=mesh,
                      in_specs=(PartitionSpec("core"),) * (n_params + n_outs),
                      out_specs=(PartitionSpec("core"),) * n_outs,
                      check_rep=False),
            donate_argnums=tuple(range(n_params, n_params + n_outs)),
            keep_unused=True,
        )
        # Donated output buffers built on device (uploading host zeros would
        # cost another ~8 MB of wire per call).
        zero_shapes = [(NCORES * a.shape[0], *a.shape[1:]) for a in out_avals]
        zero_dtypes = [a.dtype for a in out_avals]
        sh = NamedSharding(mesh, PartitionSpec("core"))
        zeros_fn = jax.jit(
            lambda: tuple(jnp.zeros(s, d)
                          for s, d in zip(zero_shapes, zero_dtypes)),
            out_shardings=(sh,) * n_outs,
        )
        _STATE = {
            "in_names": in_names,
            "out_names": out_names,
            "sharded": sharded,
            "zeros_fn": zeros_fn,
            "dbg_zero": dbg_zero,
        }
    return _STATE


def kernel(x: np.ndarray, conv_w: np.ndarray, conv_b: np.ndarray) -> np.ndarray:
    st = _get_state()
    xi, scl, w8k, w8q, obk = _prep(x, conv_w, conv_b)
    params = {
        "xi": xi.reshape(B * CT, 128, NP),
        "scl": scl.reshape(B * 128, CT),
        "w8k": np.broadcast_to(w8k, (B, 128, 6 * 256)).reshape(B * 128, 6 * 256),
        "w8q": np.broadcast_to(w8q, (B, 128, 6 * 256)).reshape(B * 128, 6 * 256),
        "obk": np.broadcast_to(obk, (B, 1, 1280)).reshape(B, 1280),
    }
    for name, z in st["dbg_zero"].items():
        params[name] = np.broadcast_to(z, (B * z.shape[0], z.shape[1]))
    zeros = st["zeros_fn"]()
    outs = st["sharded"](*[np.ascontiguousarray(params[n])
                           for n in st["in_names"]], *zeros)
    out_map = dict(zip(st["out_names"], outs))
    for o in outs:
        o.copy_to_host_async()
    og = np.asarray(out_map["og"]).reshape(B, C, N)
    osc = np.asarray(out_map["os"]).reshape(B, 128, CT).transpose(0, 2, 1)
    out = og * osc.reshape(B, C, 1).astype(np.float32)
    out += np.asarray(x, np.float32)
    return out


# revision 11
# speedup vs baseline: 5.0511x; 1.1170x over previous
"""Trainium2 Bass kernel for nn_AttentionLayer (conv1d -> linear attention -> gelu + residual).

Full inputs:  x [8, 256, 4096] f32, conv_w [512, 256, 3] f32, conv_b [512] f32
Full output:  [8, 256, 4096] f32

Sharding: pure data-parallel over batch B=8 -> 8 NeuronCores, one batch each.

The graded metric is wall-clock per kernel() call, and the axon tunnel to the
device moves ~46 MB/s with ~80 ms fixed RPC latency per dispatch, while the
on-device math is ~0.15 ms.  So this kernel is organized entirely around wire
bytes:

  UP   (11.6 MB): x int8-quantized per [b, channel] row (scale = absmax/127)
                  + f32 scales + fp8(x64) conv weights + bias consts.
  DOWN ( 8.4 MB): g = gelu(attention) int8-quantized per [b, channel] row
                  (device computes per-row absmax) + f32 scales.
  Residual "+ x" happens on the HOST, where exact f32 x is free, so neither a
  bf16 x copy (residual) nor a bf16 g needs to cross the wire.  int8 rows
  measure ~1.0e-2 end-to-end rel err (gate 2e-2): x rows are Gaussian and g
  rows have absmax/rms ~ 6, so uniform per-row quantization stays ~1% rms.

Everything else the math needs is derived on device from the int8 x:
  xb   bf16 = int8 x * row scale       (ACT copy, per-partition scale operand)
  x8   fp8  = xb                       (DVE copy; conv rhs/lhsT, DoubleRow)
  x8s  fp8  = xb shifted one column    (t=1 conv tap; dual-fp8 needs even offsets)
  vT   bf16 = x^T, 64 PE 128x128 transposes via identity matmul
The conv matmuls (75% of FLOPs) run fp8 E4M3 at 2x PE rate with weights
host-scaled by 64 (else subnormal); the 1/64 and phi's "+1" fold into the phi
chain:  with raw = 64*(conv + bias + 1) in PSUM,
  64*phi = max(raw, min(64*exp(raw/64 + ln64 - 1), 64))
so ACT does one exp straight from PSUM and DVE one fused min/max.  kv and
q@(kv) stay bf16 (kv entries get no averaging benefit from fp8).

Per-core math (C=256, N=4096, one batch):
  y  = conv1d(x, w, pad=1) + b            # [2C, N]
  q  = phi(y[:C]), k = phi(y[C:])         # phi = elu+1
  kv = k^T @ x^T                          # [C, C]   (v = x)
  g  = gelu(q @ kv)                       # [C, N]   -> int8 rows + scales
  (host) out = g * scale + x

The runner dispatches one cached pjit (shard_map over 8 cores) per call --
rebuilding it per call (as bass_utils.run_bass_kernel_spmd does) re-traces and
re-dispatches ~0.2 s of XLA work, and its donated output buffers would upload
another 8 MB of host zeros; here the donated buffers are created device-side.
"""

import numpy as np
import ml_dtypes

import jax
import jax.numpy as jnp
from jax.sharding import Mesh, NamedSharding, PartitionSpec

import concourse.bass as bass
import concourse.mybir as mybir
import concourse.tile as tile
from concourse import bacc
from concourse.masks import make_identity

F32 = mybir.dt.float32
BF16 = mybir.dt.bfloat16
FP8 = mybir.dt.float8e4
I8 = mybir.dt.int8
AF = mybir.ActivationFunctionType
ALU = mybir.AluOpType

B, C, N = 8, 256, 4096
NCORES = 8
CT = C // 128         # 2 c-tiles (partition groups) per 256-channel dim
NJ = N // 512         # 8 column chunks of 512
NT = N // 128         # 32 n-tiles of 128
NP = N + 2            # x padded with one zero column on each side
NPP = 4112            # x8 row pitch: NP padded so the dual-fp8 outer stride
                      # stays 16B-aligned
WS = 64.0             # fp8 weight scale

BF = ml_dtypes.bfloat16
F8 = ml_dtypes.float8_e4m3


def _build_nc():
    nc = bacc.Bacc("TRN2", target_bir_lowering=False, debug=False, num_devices=NCORES)

    xi_d = nc.declare_dram_parameter("xi", [CT, 128, NP], I8, isOutput=False)
    scl_d = nc.declare_dram_parameter("scl", [128, CT], F32, isOutput=False)
    # Conv weights are identical on all cores: upload 1/8 per core and
    # AllGather on-device (saves 2.75 MB of the ~46 MB/s host wire).
    wsh_d = nc.declare_dram_parameter("wsh", [16, 2, 6 * 256], FP8, isOutput=False)
    obk_d = nc.declare_dram_parameter("obk", [1, 1280], BF16, isOutput=False)
    # collectives may not touch IO tensors: bounce the shard to Internal DRAM
    wsh_t = nc.dram_tensor("wshint", [16, 2, 6 * 256], FP8)
    wg_t = nc.dram_tensor("wgather", [128, 2, 6 * 256], FP8)
    og_d = nc.declare_dram_parameter("og", [C, N], I8, isOutput=True)
    os_d = nc.declare_dram_parameter("os", [128, CT], F32, isOutput=True)

    with tile.TileContext(nc) as tc:
        with (
            tc.tile_pool(name="persist", bufs=1) as per,
            tc.tile_pool(name="tmp", bufs=4) as tmp,
            tc.tile_pool(name="psum", bufs=4, space="PSUM") as ps,
            tc.tile_pool(name="psum2", bufs=2, space="PSUM") as ps2,
            tc.tile_pool(name="psumT", bufs=2, space="PSUM") as pst,
        ):
            # ---- inputs ------------------------------------------------
            obk = per.tile([1, 1280], BF16, tag="obk", name="obk")
            nc.sync.dma_start(out=obk, in_=obk_d[:, :])
            ones128 = obk[0:1, 0:128]
            ones512 = obk[0:1, 0:512]
            bk2 = obk[0:1, 512:1024]
            scl = per.tile([128, CT], F32, tag="scl", name="scl")
            nc.sync.dma_start(out=scl, in_=scl_d[:, :])
            xi = [per.tile([128, NP], I8, tag=f"xi{ci}", name=f"xi{ci}")
                  for ci in range(CT)]
            for ci in range(CT):
                nc.sync.dma_start(out=xi[ci], in_=xi_d[ci, :, :])
            nc.sync.dma_start(out=wsh_t[:, :, :], in_=wsh_d[:, :, :])
            nc.gpsimd.collective_compute(
                "AllGather", ALU.bypass,
                replica_groups=[list(range(NCORES))],
                ins=[wsh_t[:, :, :]],
                outs=[wg_t[:, :, :]],
            )
            w8k = per.tile([128, 3, 2, 256], FP8, tag="w8k", name="w8k")
            nc.sync.dma_start(
                out=w8k,
                in_=wg_t[:, 0, :].rearrange("p (t i c) -> p t i c", i=2, c=256))
            w8q = per.tile([128, 3, 2, 256], FP8, tag="w8q", name="w8q")
            nc.sync.dma_start(
                out=w8q,
                in_=wg_t[:, 1, :].rearrange("p (t i c) -> p t i c", i=2, c=256))

            # Warm the ACT Exp table early (must read initialized SBUF).
            warm = tmp.tile([1, 1], F32, tag="warm", name="warm")
            nc.scalar.activation(warm, obk[0:1, 0:1], AF.Exp)
            # exp bias const ln(64)-1 (phi stored x64, clamp moved after exp)
            bconst = per.tile([128, 1], F32, tag="bconst", name="bconst")
            nc.gpsimd.memset(bconst, 3.1588830833596715)
            ident = per.tile([128, 128], BF16, tag="ident", name="ident")
            make_identity(nc, ident)

            # ---- derive xb (bf16), x8/x8s (fp8), vT (x^T bf16) ---------
            xb = [per.tile([128, NP], BF16, tag=f"xb{ci}", name=f"xb{ci}")
                  for ci in range(CT)]
            for ci in range(CT):
                nc.scalar.activation(xb[ci], xi[ci], AF.Copy,
                                     scale=scl[:, ci:ci + 1])
            x8 = per.tile([128, CT, NPP], FP8, tag="x8", name="x8")
            x8s = per.tile([128, CT, NPP], FP8, tag="x8s", name="x8s")
            for ci in range(CT):
                # tail cols [NP:NPP) are never read by the conv taps
                nc.vector.tensor_copy(x8[:, ci, 0:NP], xb[ci])
                nc.vector.tensor_copy(x8s[:, ci, 0:NP - 1], xb[ci][:, 1:NP])

            vT = per.tile([128, NT, 256], BF16, tag="vT", name="vT")
            for i in range(NT):
                for ci in range(CT):
                    psT = pst.tile([128, 128], BF16, tag="psT", name="psT")
                    nc.tensor.transpose(
                        psT, xb[ci][:, 1 + i * 128:1 + (i + 1) * 128], ident)
                    nc.vector.tensor_copy(vT[:, i, ci * 128:(ci + 1) * 128], psT)

            # ---- persistent intermediates ------------------------------
            kT = per.tile([128, NT, 256], BF16, tag="kT", name="kT")
            qphi = [per.tile([128, N], BF16, tag=f"qphi{ct}", name=f"qphi{ct}")
                    for ct in range(CT)]
            kv_sb = per.tile([128, CT, 256], BF16, tag="kv", name="kv_sb")

            # ---- phase NT: k^T conv (transposed layout) + fused kv -----
            # Two adjacent n-tiles share one PSUM bank and one phi chain.
            kv_ps = [ps2.tile([128, 256], F32, tag="kvp", name=f"kv_ps{ch}")
                     for ch in range(CT)]
            for ip in range(NT // 2):
                kt_ps = ps.tile([128, 512], F32, tag="bank", name="kt_ps")
                nc.tensor.matmul(kt_ps, ones128, bk2, start=True, stop=False)
                for h in range(2):
                    off = (ip * 2 + h) * 128
                    half = kt_ps[:, h * 256:(h + 1) * 256]
                    for t, (src_t, o) in enumerate(
                            ((x8, 0), (x8s, 0), (x8, 2))):
                        nc.tensor.matmul(
                            half,
                            src_t[:, :, off + o:off + o + 128],
                            w8k[:, t, :, :],
                            start=False,
                            stop=(t == 2),
                            perf_mode=mybir.MatmulPerfMode.DoubleRow,
                        )
                # 64*phi = max(raw, min(64*exp(raw/64 + ln64 - 1), 64))
                e = tmp.tile([128, 512], F32, tag="nte", name="e_nt")
                nc.scalar.activation(
                    e, kt_ps, AF.Exp, scale=1.0 / 64.0, bias=bconst[:, 0:1])
                nc.vector.scalar_tensor_tensor(
                    kT[:, ip * 2:ip * 2 + 2, :].rearrange("p i d -> p (i d)"),
                    e, 64.0, kt_ps, ALU.min, ALU.max)
                for ch in range(CT):
                    for ii in (ip * 2, ip * 2 + 1):
                        nc.tensor.matmul(
                            kv_ps[ch],
                            kT[:, ii, ch * 128:(ch + 1) * 128],
                            vT[:, ii, :],
                            start=(ii == 0),
                            stop=(ii == NT - 1),
                        )
            for ch in range(CT):
                nc.vector.tensor_copy(kv_sb[:, ch, :], kv_ps[ch])

            # ---- phase Q: conv q in [c, n] layout ----------------------
            for ct in range(CT):
                bq64 = obk[0:1, 1024 + ct * 128:1024 + (ct + 1) * 128]
                for j in range(NJ):
                    q_ps = ps.tile([128, 512], F32, tag="bank", name="q_ps")
                    nc.tensor.matmul(q_ps, bq64, ones512, start=True, stop=False)
                    for t, (src_t, o) in enumerate(
                            ((x8, 0), (x8s, 0), (x8, 2))):
                        nc.tensor.matmul(
                            q_ps,
                            w8q[:, t, :, ct * 128:(ct + 1) * 128],
                            src_t[:, :, j * 512 + o:j * 512 + o + 512],
                            start=False,
                            stop=(t == 2),
                            perf_mode=mybir.MatmulPerfMode.DoubleRow,
                        )
                    e = tmp.tile([128, 512], F32, tag="qte", name="e_q")
                    nc.scalar.activation(
                        e, q_ps, AF.Exp, scale=1.0 / 64.0, bias=bconst[:, 0:1])
                    nc.vector.scalar_tensor_tensor(
                        qphi[ct][:, j * 512:(j + 1) * 512],
                        e, 64.0, q_ps, ALU.min, ALU.max)

            # ---- phase OUT: g = gelu(q@kv), int8 rows + scales ---------
            os_sb = per.tile([128, CT], F32, tag="os", name="os_sb")
            for dt in range(CT):
                gb = per.tile([128, N], BF16, tag=f"gb{dt}", name=f"gb{dt}")
                for j in range(NJ):
                    o_ps = ps.tile([128, 512], F32, tag="bank", name="o_ps")
                    for ch in range(CT):
                        nc.tensor.matmul(
                            o_ps,
                            kv_sb[:, ch, dt * 128:(dt + 1) * 128],
                            qphi[ch][:, j * 512:(j + 1) * 512],
                            start=(ch == 0),
                            stop=(ch == CT - 1),
                        )
                    nc.scalar.activation(gb[:, j * 512:(j + 1) * 512], o_ps,
                                         AF.Gelu, scale=1.0 / 4096.0)
                gm = tmp.tile([128, 1], F32, tag="gm", name="gm")
                nc.vector.tensor_reduce(gm, gb, mybir.AxisListType.X, ALU.max,
                                        apply_absolute_value=True)
                nc.vector.tensor_scalar(gm, gm, 1e-30, None, ALU.max)
                inv = tmp.tile([128, 1], F32, tag="inv", name="inv")
                nc.vector.reciprocal(inv, gm)
                og = per.tile([128, N], I8, tag=f"og{dt}", name=f"og{dt}")
                nc.vector.tensor_scalar(og, gb, inv[:, 0:1], 127.0,
                                        ALU.mult, ALU.mult)
                nc.vector.tensor_scalar(os_sb[:, dt:dt + 1], gm, 1.0 / 127.0,
                                        None, ALU.mult)
                nc.sync.dma_start(out=og_d[dt * 128:(dt + 1) * 128, :], in_=og)
            nc.sync.dma_start(out=os_d[:, :], in_=os_sb)

    nc.compile()
    return nc


def _prep(x, conv_w, conv_b):
    x = np.asarray(x, dtype=np.float32)
    conv_w = np.asarray(conv_w, dtype=np.float32)
    conv_b = np.asarray(conv_b, dtype=np.float32)

    # int8 per-[b, channel] row quantization of x (absmax -> +-127)
    rs = np.abs(x).max(axis=2)
    np.maximum(rs, 1e-30, out=rs)
    t = np.multiply(x, (127.0 / rs)[:, :, None])
    np.rint(t, out=t)
    global _XI_BUF
    if _XI_BUF is None:
        _XI_BUF = np.zeros((B, CT, 128, NP), dtype=np.int8)
    xi = _XI_BUF
    xi[:, :, :, 1:N + 1] = t.astype(np.int8).reshape(B, CT, 128, N)
    scl = np.ascontiguousarray(
        (rs * (1.0 / 127.0)).reshape(B, CT, 128).transpose(0, 2, 1))

    # conv weights x64 in fp8, DoubleRow slot layout [p, t, ci, co]
    w = conv_w.transpose(2, 1, 0).reshape(3, CT, 128, 2 * C)
    w = w.transpose(1, 0, 2, 3)                      # [ci, t, p, co]
    w8q = np.ascontiguousarray(
        w[:, :, :, :C].transpose(2, 1, 0, 3) * WS).reshape(128, 6 * 256).astype(F8)
    w8k = np.ascontiguousarray(
        w[:, :, :, C:].transpose(2, 1, 0, 3) * WS).reshape(128, 6 * 256).astype(F8)
    obk = np.ones((1, 1280), dtype=np.float32)
    obk[0, 512:768] = WS * (conv_b[C:] + 1.0)
    obk[0, 768:1024] = WS * (conv_b[C:] + 1.0)
    obk[0, 1024:1280] = WS * (conv_b[:C] + 1.0)
    obk = obk.astype(BF)
    return xi, scl, w8k, w8q, obk


_STATE = None
_XI_BUF = None


def _get_state():
    global _STATE
    if _STATE is None:
        from concourse.bass2jax import (
            _bass_exec_p, install_neuronx_cc_hook, partition_id_tensor)
        from jax.experimental.shard_map import shard_map

        nc = _build_nc()
        install_neuronx_cc_hook()

        partition_name = (nc.partition_id_tensor.name
                          if nc.partition_id_tensor else None)
        in_names, out_names, out_avals = [], [], []
        for alloc in nc.m.functions[0].allocations:
            if not isinstance(alloc, mybir.MemoryLocationSet):
                continue
            name = alloc.memorylocations[0].name
            if alloc.kind == "ExternalInput":
                if name != partition_name:
                    in_names.append(name)
            elif alloc.kind == "ExternalOutput":
                out_names.append(name)
                out_avals.append(jax.core.ShapedArray(
                    tuple(alloc.tensor_shape), mybir.dt.np(alloc.dtype)))
        dbg_zero = {}
        if nc.dbg_addr is not None:
            dbg_zero = {nc.dbg_addr.name: np.zeros((1, 2), np.uint32)}
            if nc.dbg_addr.name not in in_names:
                in_names.append(nc.dbg_addr.name)
        n_params = len(in_names)
        n_outs = len(out_names)
        all_names = in_names + out_names
        if partition_name is not None:
            all_names.append(partition_name)

        def _body(*args):
            operands = list(args)
            if partition_name is not None:
                operands.append(partition_id_tensor())
            return tuple(_bass_exec_p.bind(
                *operands,
                out_avals=tuple(out_avals),
                in_names=tuple(all_names),
                out_names=tuple(out_names),
                lowering_input_output_aliases=(),
                sim_require_finite=True,
                sim_require_nnan=True,
                nc=nc,
            ))

        devices = jax.devices()[:NCORES]
        mesh = Mesh(np.asarray(devices), ("core",))
        sharded = jax.jit(
            shard_map(_body, mesh=mesh,
                      in_specs=(PartitionSpec("core"),) * (n_params + n_outs),
                      out_specs=(PartitionSpec("core"),) * n_outs,
                      check_rep=False),
            donate_argnums=tuple(range(n_params, n_params + n_outs)),
            keep_unused=True,
        )
        # Donated output buffers built on device (uploading host zeros would
        # cost another ~8 MB of wire per call).
        zero_shapes = [(NCORES * a.shape[0], *a.shape[1:]) for a in out_avals]
        zero_dtypes = [a.dtype for a in out_avals]
        sh = NamedSharding(mesh, PartitionSpec("core"))
        zeros_fn = jax.jit(
            lambda: tuple(jnp.zeros(s, d)
                          for s, d in zip(zero_shapes, zero_dtypes)),
            out_shardings=(sh,) * n_outs,
        )
        _STATE = {
            "in_names": in_names,
            "out_names": out_names,
            "sharded": sharded,
            "zeros_fn": zeros_fn,
            "dbg_zero": dbg_zero,
        }
    return _STATE


def kernel(x: np.ndarray, conv_w: np.ndarray, conv_b: np.ndarray) -> np.ndarray:
    st = _get_state()
    xi, scl, w8k, w8q, obk = _prep(x, conv_w, conv_b)
    params = {
        "xi": xi.reshape(B * CT, 128, NP),
        "scl": scl.reshape(B * 128, CT),
        # [128, 2, 1536] sliced into 8 x [16, 2, 1536] core shards = itself
        "wsh": np.stack([w8k, w8q], axis=1),
        "obk": np.broadcast_to(obk, (B, 1, 1280)).reshape(B, 1280),
    }
    for name, z in st["dbg_zero"].items():
        params[name] = np.broadcast_to(z, (B * z.shape[0], z.shape[1]))
    zeros = st["zeros_fn"]()
    outs = st["sharded"](*[np.ascontiguousarray(params[n])
                           for n in st["in_names"]], *zeros)
    out_map = dict(zip(st["out_names"], outs))
    for o in outs:
        o.copy_to_host_async()
    og = np.asarray(out_map["og"]).reshape(B, C, N)
    osc = np.asarray(out_map["os"]).reshape(B, 128, CT).transpose(0, 2, 1)
    out = og * osc.reshape(B, C, 1).astype(np.float32)
    out += np.asarray(x, np.float32)
    return out


# revision 14
# speedup vs baseline: 5.9221x; 1.1724x over previous
"""Trainium2 Bass kernel for nn_AttentionLayer (conv1d -> linear attention -> gelu + residual).

Full inputs:  x [8, 256, 4096] f32, conv_w [512, 256, 3] f32, conv_b [512] f32
Full output:  [8, 256, 4096] f32

Sharding: pure data-parallel over batch B=8 -> 8 NeuronCores, one batch each.

The graded metric is wall-clock per kernel() call, and the axon tunnel to the
device moves ~46 MB/s with ~80 ms fixed RPC latency per dispatch, while the
on-device math is ~0.15 ms.  So this kernel is organized entirely around wire
bytes:

  UP   (11.6 MB): x int8-quantized per [b, channel] row (scale = absmax/127)
                  + f32 scales + fp8(x64) conv weights + bias consts.
  DOWN ( 8.4 MB): g = gelu(attention) int8-quantized per [b, channel] row
                  (device computes per-row absmax) + f32 scales.
  Residual "+ x" happens on the HOST, where exact f32 x is free, so neither a
  bf16 x copy (residual) nor a bf16 g needs to cross the wire.  int8 rows
  measure ~1.0e-2 end-to-end rel err (gate 2e-2): x rows are Gaussian and g
  rows have absmax/rms ~ 6, so uniform per-row quantization stays ~1% rms.

Everything else the math needs is derived on device from the int8 x:
  xb   bf16 = int8 x * row scale       (ACT copy, per-partition scale operand)
  x8   fp8  = xb                       (DVE copy; conv rhs/lhsT, DoubleRow)
  x8s  fp8  = xb shifted one column    (t=1 conv tap; dual-fp8 needs even offsets)
  vT   bf16 = x^T, 64 PE 128x128 transposes via identity matmul
The conv matmuls (75% of FLOPs) run fp8 E4M3 at 2x PE rate with weights
host-scaled by 64 (else subnormal); the 1/64 and phi's "+1" fold into the phi
chain:  with raw = 64*(conv + bias + 1) in PSUM,
  64*phi = max(raw, min(64*exp(raw/64 + ln64 - 1), 64))
so ACT does one exp straight from PSUM and DVE one fused min/max.  kv and
q@(kv) stay bf16 (kv entries get no averaging benefit from fp8).

Per-core math (C=256, N=4096, one batch):
  y  = conv1d(x, w, pad=1) + b            # [2C, N]
  q  = phi(y[:C]), k = phi(y[C:])         # phi = elu+1
  kv = k^T @ x^T                          # [C, C]   (v = x)
  g  = gelu(q @ kv)                       # [C, N]   -> int8 rows + scales
  (host) out = g * scale + x

The runner dispatches one cached pjit (shard_map over 8 cores) per call --
rebuilding it per call (as bass_utils.run_bass_kernel_spmd does) re-traces and
re-dispatches ~0.2 s of XLA work, and its donated output buffers would upload
another 8 MB of host zeros; here the donated buffers are created device-side.
"""

import numba
import numpy as np
import ml_dtypes

import jax
import jax.numpy as jnp
from jax.sharding import Mesh, NamedSharding, PartitionSpec

import concourse.bass as bass
import concourse.mybir as mybir
import concourse.tile as tile
from concourse import bacc
from concourse.masks import make_identity

F32 = mybir.dt.float32
BF16 = mybir.dt.bfloat16
FP8 = mybir.dt.float8e4
I8 = mybir.dt.int8
AF = mybir.ActivationFunctionType
ALU = mybir.AluOpType

B, C, N = 8, 256, 4096
NCORES = 8
CT = C // 128         # 2 c-tiles (partition groups) per 256-channel dim
NJ = N // 512         # 8 column chunks of 512
NT = N // 128         # 32 n-tiles of 128
NP = N + 2            # x padded with one zero column on each side
NPP = 4112            # x8 row pitch: NP padded so the dual-fp8 outer stride
                      # stays 16B-aligned
WS = 64.0             # fp8 weight scale

BF = ml_dtypes.bfloat16
F8 = ml_dtypes.float8_e4m3


def _build_nc():
    nc = bacc.Bacc("TRN2", target_bir_lowering=False, debug=False, num_devices=NCORES)

    xi_d = nc.declare_dram_parameter("xi", [CT, 128, NP], I8, isOutput=False)
    scl_d = nc.declare_dram_parameter("scl", [128, CT], F32, isOutput=False)
    # Conv weights are identical on all cores: upload 1/8 per core and
    # AllGather on-device (saves 2.75 MB of the ~46 MB/s host wire).
    wsh_d = nc.declare_dram_parameter("wsh", [16, 2, 6 * 256], FP8, isOutput=False)
    obk_d = nc.declare_dram_parameter("obk", [1, 1280], BF16, isOutput=False)
    # collectives may not touch IO tensors: bounce the shard to Internal DRAM
    wsh_t = nc.dram_tensor("wshint", [16, 2, 6 * 256], FP8)
    wg_t = nc.dram_tensor("wgather", [128, 2, 6 * 256], FP8)
    og_d = nc.declare_dram_parameter("og", [C, N], I8, isOutput=True)
    os_d = nc.declare_dram_parameter("os", [128, CT], F32, isOutput=True)

    with tile.TileContext(nc) as tc:
        with (
            tc.tile_pool(name="persist", bufs=1) as per,
            tc.tile_pool(name="tmp", bufs=4) as tmp,
            tc.tile_pool(name="psum", bufs=4, space="PSUM") as ps,
            tc.tile_pool(name="psum2", bufs=2, space="PSUM") as ps2,
            tc.tile_pool(name="psumT", bufs=2, space="PSUM") as pst,
        ):
            # ---- inputs ------------------------------------------------
            obk = per.tile([1, 1280], BF16, tag="obk", name="obk")
            nc.sync.dma_start(out=obk, in_=obk_d[:, :])
            ones128 = obk[0:1, 0:128]
            ones512 = obk[0:1, 0:512]
            bk2 = obk[0:1, 512:1024]
            scl = per.tile([128, CT], F32, tag="scl", name="scl")
            nc.sync.dma_start(out=scl, in_=scl_d[:, :])
            xi = [per.tile([128, NP], I8, tag=f"xi{ci}", name=f"xi{ci}")
                  for ci in range(CT)]
            for ci in range(CT):
                nc.sync.dma_start(out=xi[ci], in_=xi_d[ci, :, :])
            nc.sync.dma_start(out=wsh_t[:, :, :], in_=wsh_d[:, :, :])
            nc.gpsimd.collective_compute(
                "AllGather", ALU.bypass,
                replica_groups=[list(range(NCORES))],
                ins=[wsh_t[:, :, :]],
                outs=[wg_t[:, :, :]],
            )
            w8k = per.tile([128, 3, 2, 256], FP8, tag="w8k", name="w8k")
            nc.sync.dma_start(
                out=w8k,
                in_=wg_t[:, 0, :].rearrange("p (t i c) -> p t i c", i=2, c=256))
            w8q = per.tile([128, 3, 2, 256], FP8, tag="w8q", name="w8q")
            nc.sync.dma_start(
                out=w8q,
                in_=wg_t[:, 1, :].rearrange("p (t i c) -> p t i c", i=2, c=256))

            # Warm the ACT Exp table early (must read initialized SBUF).
            warm = tmp.tile([1, 1], F32, tag="warm", name="warm")
            nc.scalar.activation(warm, obk[0:1, 0:1], AF.Exp)
            # exp bias const ln(64)-1 (phi stored x64, clamp moved after exp)
            bconst = per.tile([128, 1], F32, tag="bconst", name="bconst")
            nc.gpsimd.memset(bconst, 3.1588830833596715)
            ident = per.tile([128, 128], BF16, tag="ident", name="ident")
            make_identity(nc, ident)

            # ---- derive xb (bf16), x8/x8s (fp8), vT (x^T bf16) ---------
            xb = [per.tile([128, NP], BF16, tag=f"xb{ci}", name=f"xb{ci}")
                  for ci in range(CT)]
            for ci in range(CT):
                nc.scalar.activation(xb[ci], xi[ci], AF.Copy,
                                     scale=scl[:, ci:ci + 1])
            x8 = per.tile([128, CT, NPP], FP8, tag="x8", name="x8")
            x8s = per.tile([128, CT, NPP], FP8, tag="x8s", name="x8s")
            for ci in range(CT):
                # tail cols [NP:NPP) are never read by the conv taps
                nc.vector.tensor_copy(x8[:, ci, 0:NP], xb[ci])
                nc.vector.tensor_copy(x8s[:, ci, 0:NP - 1], xb[ci][:, 1:NP])

            vT = per.tile([128, NT, 256], BF16, tag="vT", name="vT")
            for i in range(NT):
                for ci in range(CT):
                    psT = pst.tile([128, 128], BF16, tag="psT", name="psT")
                    nc.tensor.transpose(
                        psT, xb[ci][:, 1 + i * 128:1 + (i + 1) * 128], ident)
                    nc.vector.tensor_copy(vT[:, i, ci * 128:(ci + 1) * 128], psT)

            # ---- persistent intermediates ------------------------------
            kT = per.tile([128, NT, 256], BF16, tag="kT", name="kT")
            qphi = [per.tile([128, N], BF16, tag=f"qphi{ct}", name=f"qphi{ct}")
                    for ct in range(CT)]
            kv_sb = per.tile([128, CT, 256], BF16, tag="kv", name="kv_sb")

            # ---- phase NT: k^T conv (transposed layout) + fused kv -----
            # Two adjacent n-tiles share one PSUM bank and one phi chain.
            kv_ps = [ps2.tile([128, 256], F32, tag="kvp", name=f"kv_ps{ch}")
                     for ch in range(CT)]
            for ip in range(NT // 2):
                kt_ps = ps.tile([128, 512], F32, tag="bank", name="kt_ps")
                nc.tensor.matmul(kt_ps, ones128, bk2, start=True, stop=False)
                for h in range(2):
                    off = (ip * 2 + h) * 128
                    half = kt_ps[:, h * 256:(h + 1) * 256]
                    for t, (src_t, o) in enumerate(
                            ((x8, 0), (x8s, 0), (x8, 2))):
                        nc.tensor.matmul(
                            half,
                            src_t[:, :, off + o:off + o + 128],
                            w8k[:, t, :, :],
                            start=False,
                            stop=(t == 2),
                            perf_mode=mybir.MatmulPerfMode.DoubleRow,
                        )
                # 64*phi = max(raw, min(64*exp(raw/64 + ln64 - 1), 64))
                e = tmp.tile([128, 512], F32, tag="nte", name="e_nt")
                nc.scalar.activation(
                    e, kt_ps, AF.Exp, scale=1.0 / 64.0, bias=bconst[:, 0:1])
                nc.vector.scalar_tensor_tensor(
                    kT[:, ip * 2:ip * 2 + 2, :].rearrange("p i d -> p (i d)"),
                    e, 64.0, kt_ps, ALU.min, ALU.max)
                for ch in range(CT):
                    for ii in (ip * 2, ip * 2 + 1):
                        nc.tensor.matmul(
                            kv_ps[ch],
                            kT[:, ii, ch * 128:(ch + 1) * 128],
                            vT[:, ii, :],
                            start=(ii == 0),
                            stop=(ii == NT - 1),
                        )
            for ch in range(CT):
                nc.vector.tensor_copy(kv_sb[:, ch, :], kv_ps[ch])

            # ---- phase Q: conv q in [c, n] layout ----------------------
            for ct in range(CT):
                bq64 = obk[0:1, 1024 + ct * 128:1024 + (ct + 1) * 128]
                for j in range(NJ):
                    q_ps = ps.tile([128, 512], F32, tag="bank", name="q_ps")
                    nc.tensor.matmul(q_ps, bq64, ones512, start=True, stop=False)
                    for t, (src_t, o) in enumerate(
                            ((x8, 0), (x8s, 0), (x8, 2))):
                        nc.tensor.matmul(
                            q_ps,
                            w8q[:, t, :, ct * 128:(ct + 1) * 128],
                            src_t[:, :, j * 512 + o:j * 512 + o + 512],
                            start=False,
                            stop=(t == 2),
                            perf_mode=mybir.MatmulPerfMode.DoubleRow,
                        )
                    e = tmp.tile([128, 512], F32, tag="qte", name="e_q")
                    nc.scalar.activation(
                        e, q_ps, AF.Exp, scale=1.0 / 64.0, bias=bconst[:, 0:1])
                    nc.vector.scalar_tensor_tensor(
                        qphi[ct][:, j * 512:(j + 1) * 512],
                        e, 64.0, q_ps, ALU.min, ALU.max)

            # ---- phase OUT: g = gelu(q@kv), int8 rows + scales ---------
            os_sb = per.tile([128, CT], F32, tag="os", name="os_sb")
            for dt in range(CT):
                gb = per.tile([128, N], BF16, tag=f"gb{dt}", name=f"gb{dt}")
                for j in range(NJ):
                    o_ps = ps.tile([128, 512], F32, tag="bank", name="o_ps")
                    for ch in range(CT):
                        nc.tensor.matmul(
                            o_ps,
                            kv_sb[:, ch, dt * 128:(dt + 1) * 128],
                            qphi[ch][:, j * 512:(j + 1) * 512],
                            start=(ch == 0),
                            stop=(ch == CT - 1),
                        )
                    nc.scalar.activation(gb[:, j * 512:(j + 1) * 512], o_ps,
                                         AF.Gelu, scale=1.0 / 4096.0)
                gm = tmp.tile([128, 1], F32, tag="gm", name="gm")
                nc.vector.tensor_reduce(gm, gb, mybir.AxisListType.X, ALU.max,
                                        apply_absolute_value=True)
                nc.vector.tensor_scalar(gm, gm, 1e-30, None, ALU.max)
                inv = tmp.tile([128, 1], F32, tag="inv", name="inv")
                nc.vector.reciprocal(inv, gm)
                og = per.tile([128, N], I8, tag=f"og{dt}", name=f"og{dt}")
                nc.vector.tensor_scalar(og, gb, inv[:, 0:1], 127.0,
                                        ALU.mult, ALU.mult)
                nc.vector.tensor_scalar(os_sb[:, dt:dt + 1], gm, 1.0 / 127.0,
                                        None, ALU.mult)
                nc.sync.dma_start(out=og_d[dt * 128:(dt + 1) * 128, :], in_=og)
            nc.sync.dma_start(out=os_d[:, :], in_=os_sb)

    nc.compile()
    return nc


@numba.njit(fastmath=True)
def _quant_rows(x, out, scale):
    # per-row absmax -> int8 in [:, 1:N+1] (cols 0 / N+1 stay zero padding)
    rows, n = x.shape
    for r in range(rows):
        m = 0.0
        for i in range(n):
            m = max(m, abs(x[r, i]))
        m = max(m, 1e-30)
        s = 127.0 / m
        for i in range(n):
            out[r, i + 1] = np.int8(np.rint(x[r, i] * s))
        scale[r] = m * (1.0 / 127.0)


@numba.njit(fastmath=True)
def _dequant_rows(og, osc, x, out):
    rows, n = og.shape
    for r in range(rows):
        s = osc[r]
        for i in range(n):
            out[r, i] = og[r, i] * s + x[r, i]


def _prep(x, conv_w, conv_b):
    x = np.asarray(x, dtype=np.float32)
    conv_w = np.asarray(conv_w, dtype=np.float32)
    conv_b = np.asarray(conv_b, dtype=np.float32)

    # int8 per-[b, channel] row quantization of x (absmax -> +-127)
    global _XI_BUF
    if _XI_BUF is None:
        _XI_BUF = np.zeros((B, CT, 128, NP), dtype=np.int8)
    xi = _XI_BUF
    rs = np.empty((B, C), dtype=np.float32)
    _quant_rows(x.reshape(B * C, N), xi.reshape(B * C, NP), rs.reshape(B * C))
    scl = np.ascontiguousarray(rs.reshape(B, CT, 128).transpose(0, 2, 1))

    # conv weights x64 in fp8, DoubleRow slot layout [p, t, ci, co]
    w = conv_w.transpose(2, 1, 0).reshape(3, CT, 128, 2 * C)
    w = w.transpose(1, 0, 2, 3)                      # [ci, t, p, co]
    w8q = np.ascontiguousarray(
        w[:, :, :, :C].transpose(2, 1, 0, 3) * WS).reshape(128, 6 * 256).astype(F8)
    w8k = np.ascontiguousarray(
        w[:, :, :, C:].transpose(2, 1, 0, 3) * WS).reshape(128, 6 * 256).astype(F8)
    obk = np.ones((1, 1280), dtype=np.float32)
    obk[0, 512:768] = WS * (conv_b[C:] + 1.0)
    obk[0, 768:1024] = WS * (conv_b[C:] + 1.0)
    obk[0, 1024:1280] = WS * (conv_b[:C] + 1.0)
    obk = obk.astype(BF)
    return xi, scl, w8k, w8q, obk


_STATE = None
_XI_BUF = None


def _get_state():
    global _STATE
    if _STATE is None:
        from concourse.bass2jax import (
            _bass_exec_p, install_neuronx_cc_hook, partition_id_tensor)
        from jax.experimental.shard_map import shard_map

        nc = _build_nc()
        install_neuronx_cc_hook()

        partition_name = (nc.partition_id_tensor.name
                          if nc.partition_id_tensor else None)
        in_names, out_names, out_avals = [], [], []
        for alloc in nc.m.functions[0].allocations:
            if not isinstance(alloc, mybir.MemoryLocationSet):
                continue
            name = alloc.memorylocations[0].name
            if alloc.kind == "ExternalInput":
                if name != partition_name:
                    in_names.append(name)
            elif alloc.kind == "ExternalOutput":
                out_names.append(name)
                out_avals.append(jax.core.ShapedArray(
                    tuple(alloc.tensor_shape), mybir.dt.np(alloc.dtype)))
        dbg_zero = {}
        if nc.dbg_addr is not None:
            dbg_zero = {nc.dbg_addr.name: np.zeros((1, 2), np.uint32)}
            if nc.dbg_addr.name not in in_names:
                in_names.append(nc.dbg_addr.name)
        n_params = len(in_names)
        n_outs = len(out_names)
        all_names = in_names + out_names
        if partition_name is not None:
            all_names.append(partition_name)

        def _body(*args):
            operands = list(args)
            if partition_name is not None:
                operands.append(partition_id_tensor())
            return tuple(_bass_exec_p.bind(
                *operands,
                out_avals=tuple(out_avals),
                in_names=tuple(all_names),
                out_names=tuple(out_names),
                lowering_input_output_aliases=(),
                sim_require_finite=True,
                sim_require_nnan=True,
                nc=nc,
            ))

        devices = jax.devices()[:NCORES]
        mesh = Mesh(np.asarray(devices), ("core",))
        sharded = jax.jit(
            shard_map(_body, mesh=mesh,
                      in_specs=(PartitionSpec("core"),) * (n_params + n_outs),
                      out_specs=(PartitionSpec("core"),) * n_outs,
                      check_rep=False),
            donate_argnums=tuple(range(n_params, n_params + n_outs)),
            keep_unused=True,
        )
        # Donated output buffers built on device (uploading host zeros would
        # cost another ~8 MB of wire per call).
        zero_shapes = [(NCORES * a.shape[0], *a.shape[1:]) for a in out_avals]
        zero_dtypes = [a.dtype for a in out_avals]
        sh = NamedSharding(mesh, PartitionSpec("core"))
        zeros_fn = jax.jit(
            lambda: tuple(jnp.zeros(s, d)
                          for s, d in zip(zero_shapes, zero_dtypes)),
            out_shardings=(sh,) * n_outs,
        )
        _STATE = {
            "in_names": in_names,
            "out_names": out_names,
            "sharded": sharded,
            "zeros_fn": zeros_fn,
            "dbg_zero": dbg_zero,
        }
    return _STATE


def kernel(x: np.ndarray, conv_w: np.ndarray, conv_b: np.ndarray) -> np.ndarray:
    st = _get_state()
    xi, scl, w8k, w8q, obk = _prep(x, conv_w, conv_b)
    params = {
        "xi": xi.reshape(B * CT, 128, NP),
        "scl": scl.reshape(B * 128, CT),
        # [128, 2, 1536] sliced into 8 x [16, 2, 1536] core shards = itself
        "wsh": np.stack([w8k, w8q], axis=1),
        "obk": np.broadcast_to(obk, (B, 1, 1280)).reshape(B, 1280),
    }
    for name, z in st["dbg_zero"].items():
        params[name] = np.broadcast_to(z, (B * z.shape[0], z.shape[1]))
    zeros = st["zeros_fn"]()
    outs = st["sharded"](*[np.ascontiguousarray(params[n])
                           for n in st["in_names"]], *zeros)
    out_map = dict(zip(st["out_names"], outs))
    for o in outs:
        o.copy_to_host_async()
    og = np.asarray(out_map["og"])
    osc = np.ascontiguousarray(
        np.asarray(out_map["os"]).reshape(B, 128, CT).transpose(0, 2, 1))
    out = np.empty((B, C, N), dtype=np.float32)
    _dequant_rows(og.reshape(B * C, N), osc.reshape(B * C),
                  np.asarray(x, np.float32).reshape(B * C, N),
                  out.reshape(B * C, N))
    return out


# revision 19
# speedup vs baseline: 6.1874x; 1.0448x over previous
"""Trainium2 Bass kernel for nn_AttentionLayer (conv1d -> linear attention -> gelu + residual).

Full inputs:  x [8, 256, 4096] f32, conv_w [512, 256, 3] f32, conv_b [512] f32
Full output:  [8, 256, 4096] f32

Sharding: pure data-parallel over batch B=8 -> 8 NeuronCores, one batch each.

The graded metric is wall-clock per kernel() call, and the axon tunnel to the
device moves ~46 MB/s with ~80 ms fixed RPC latency per dispatch, while the
on-device math is ~0.15 ms.  So this kernel is organized entirely around wire
bytes:

  UP   (11.6 MB): x int8-quantized per [b, channel] row (scale = absmax/127)
                  + f32 scales + fp8(x64) conv weights + bias consts.
  DOWN ( 8.4 MB): g = gelu(attention) int8-quantized per [b, channel] row
                  (device computes per-row absmax) + f32 scales.
  Residual "+ x" happens on the HOST, where exact f32 x is free, so neither a
  bf16 x copy (residual) nor a bf16 g needs to cross the wire.  int8 rows
  measure ~1.0e-2 end-to-end rel err (gate 2e-2): x rows are Gaussian and g
  rows have absmax/rms ~ 6, so uniform per-row quantization stays ~1% rms.

Everything else the math needs is derived on device from the int8 x:
  xb   bf16 = int8 x * row scale       (ACT copy, per-partition scale operand)
  x8   fp8  = xb                       (DVE copy; conv rhs/lhsT, DoubleRow)
  x8s  fp8  = xb shifted one column    (t=1 conv tap; dual-fp8 needs even offsets)
  vT   bf16 = x^T, 64 PE 128x128 transposes via identity matmul
The conv matmuls (75% of FLOPs) run fp8 E4M3 at 2x PE rate with weights
host-scaled by 64 (else subnormal); the 1/64 and phi's "+1" fold into the phi
chain:  with raw = 64*(conv + bias + 1) in PSUM,
  64*phi = max(raw, min(64*exp(raw/64 + ln64 - 1), 64))
so ACT does one exp straight from PSUM and DVE one fused min/max.  kv and
q@(kv) stay bf16 (kv entries get no averaging benefit from fp8).

Per-core math (C=256, N=4096, one batch):
  y  = conv1d(x, w, pad=1) + b            # [2C, N]
  q  = phi(y[:C]), k = phi(y[C:])         # phi = elu+1
  kv = k^T @ x^T                          # [C, C]   (v = x)
  g  = gelu(q @ kv)                       # [C, N]   -> int8 rows + scales
  (host) out = g * scale + x

The runner dispatches one cached pjit (shard_map over 8 cores) per call --
rebuilding it per call (as bass_utils.run_bass_kernel_spmd does) re-traces and
re-dispatches ~0.2 s of XLA work, and its donated output buffers would upload
another 8 MB of host zeros; here the donated buffers are created device-side.
"""

import numba
import numpy as np
import ml_dtypes

import jax
import jax.numpy as jnp
from jax.sharding import Mesh, NamedSharding, PartitionSpec

import concourse.bass as bass
import concourse.mybir as mybir
import concourse.tile as tile
from concourse import bacc
from concourse.masks import make_identity

F32 = mybir.dt.float32
BF16 = mybir.dt.bfloat16
FP8 = mybir.dt.float8e4
I8 = mybir.dt.int8
AF = mybir.ActivationFunctionType
ALU = mybir.AluOpType

B, C, N = 8, 256, 4096
NCORES = 8
CT = C // 128         # 2 c-tiles (partition groups) per 256-channel dim
NJ = N // 512         # 8 column chunks of 512
NT = N // 128         # 32 n-tiles of 128
NP = N + 2            # x padded with one zero column on each side
NPP = 4112            # x8 row pitch: NP padded so the dual-fp8 outer stride
                      # stays 16B-aligned
WS = 64.0             # fp8 weight scale

BF = ml_dtypes.bfloat16
F8 = ml_dtypes.float8_e4m3


def _build_nc():
    nc = bacc.Bacc("TRN2", target_bir_lowering=False, debug=False, num_devices=NCORES)

    # x int8 split into two channel-half params so the host can overlap
    # quantization of half 1 with the (async) upload of half 0
    xi0_d = nc.declare_dram_parameter("xi0", [128, NP], I8, isOutput=False)
    xi1_d = nc.declare_dram_parameter("xi1", [128, NP], I8, isOutput=False)
    scl_d = nc.declare_dram_parameter("scl", [128, CT], F32, isOutput=False)
    # Conv weights are identical on all cores: upload 1/8 per core and
    # AllGather on-device (saves 2.75 MB of the ~46 MB/s host wire).
    wsh_d = nc.declare_dram_parameter("wsh", [16, 2, 6 * 256], FP8, isOutput=False)
    obk_d = nc.declare_dram_parameter("obk", [1, 1280], BF16, isOutput=False)
    # collectives may not touch IO tensors: bounce the shard to Internal DRAM
    wsh_t = nc.dram_tensor("wshint", [16, 2, 6 * 256], FP8)
    wg_t = nc.dram_tensor("wgather", [128, 2, 6 * 256], FP8)
    og_d = nc.declare_dram_parameter("og", [C, N], I8, isOutput=True)
    os_d = nc.declare_dram_parameter("os", [128, CT], F32, isOutput=True)

    with tile.TileContext(nc) as tc:
        with (
            tc.tile_pool(name="persist", bufs=1) as per,
            tc.tile_pool(name="tmp", bufs=4) as tmp,
            tc.tile_pool(name="psum", bufs=4, space="PSUM") as ps,
            tc.tile_pool(name="psum2", bufs=2, space="PSUM") as ps2,
            tc.tile_pool(name="psumT", bufs=2, space="PSUM") as pst,
        ):
            # ---- inputs ------------------------------------------------
            obk = per.tile([1, 1280], BF16, tag="obk", name="obk")
            nc.sync.dma_start(out=obk, in_=obk_d[:, :])
            ones128 = obk[0:1, 0:128]
            ones512 = obk[0:1, 0:512]
            bk2 = obk[0:1, 512:1024]
            scl = per.tile([128, CT], F32, tag="scl", name="scl")
            nc.sync.dma_start(out=scl, in_=scl_d[:, :])
            xi = [per.tile([128, NP], I8, tag=f"xi{ci}", name=f"xi{ci}")
                  for ci in range(CT)]
            nc.sync.dma_start(out=xi[0], in_=xi0_d[:, :])
            nc.sync.dma_start(out=xi[1], in_=xi1_d[:, :])
            nc.sync.dma_start(out=wsh_t[:, :, :], in_=wsh_d[:, :, :])
            nc.gpsimd.collective_compute(
                "AllGather", ALU.bypass,
                replica_groups=[list(range(NCORES))],
                ins=[wsh_t[:, :, :]],
                outs=[wg_t[:, :, :]],
            )
            w8k = per.tile([128, 3, 2, 256], FP8, tag="w8k", name="w8k")
            nc.sync.dma_start(
                out=w8k,
                in_=wg_t[:, 0, :].rearrange("p (t i c) -> p t i c", i=2, c=256))
            w8q = per.tile([128, 3, 2, 256], FP8, tag="w8q", name="w8q")
            nc.sync.dma_start(
                out=w8q,
                in_=wg_t[:, 1, :].rearrange("p (t i c) -> p t i c", i=2, c=256))

            # Warm the ACT Exp table early (must read initialized SBUF).
            warm = tmp.tile([1, 1], F32, tag="warm", name="warm")
            nc.scalar.activation(warm, obk[0:1, 0:1], AF.Exp)
            # exp bias const ln(64)-1 (phi stored x64, clamp moved after exp)
            bconst = per.tile([128, 1], F32, tag="bconst", name="bconst")
            nc.gpsimd.memset(bconst, 3.1588830833596715)
            ident = per.tile([128, 128], BF16, tag="ident", name="ident")
            make_identity(nc, ident)

            # ---- derive xb (bf16), x8/x8s (fp8), vT (x^T bf16) ---------
            xb = [per.tile([128, NP], BF16, tag=f"xb{ci}", name=f"xb{ci}")
                  for ci in range(CT)]
            for ci in range(CT):
                nc.scalar.activation(xb[ci], xi[ci], AF.Copy,
                                     scale=scl[:, ci:ci + 1])
            x8 = per.tile([128, CT, NPP], FP8, tag="x8", name="x8")
            x8s = per.tile([128, CT, NPP], FP8, tag="x8s", name="x8s")
            for ci in range(CT):
                # tail cols [NP:NPP) are never read by the conv taps
                nc.vector.tensor_copy(x8[:, ci, 0:NP], xb[ci])
                nc.vector.tensor_copy(x8s[:, ci, 0:NP - 1], xb[ci][:, 1:NP])

            vT = per.tile([128, NT, 256], BF16, tag="vT", name="vT")
            for i in range(NT):
                for ci in range(CT):
                    psT = pst.tile([128, 128], BF16, tag="psT", name="psT")
                    nc.tensor.transpose(
                        psT, xb[ci][:, 1 + i * 128:1 + (i + 1) * 128], ident)
                    nc.vector.tensor_copy(vT[:, i, ci * 128:(ci + 1) * 128], psT)

            # ---- persistent intermediates ------------------------------
            kT = per.tile([128, NT, 256], BF16, tag="kT", name="kT")
            qphi = [per.tile([128, N], BF16, tag=f"qphi{ct}", name=f"qphi{ct}")
                    for ct in range(CT)]
            kv_sb = per.tile([128, CT, 256], BF16, tag="kv", name="kv_sb")

            # ---- phase NT: k^T conv (transposed layout) + fused kv -----
            # Two adjacent n-tiles share one PSUM bank and one phi chain.
            kv_ps = [ps2.tile([128, 256], F32, tag="kvp", name=f"kv_ps{ch}")
                     for ch in range(CT)]
            for ip in range(NT // 2):
                kt_ps = ps.tile([128, 512], F32, tag="bank", name="kt_ps")
                nc.tensor.matmul(kt_ps, ones128, bk2, start=True, stop=False)
                for h in range(2):
                    off = (ip * 2 + h) * 128
                    half = kt_ps[:, h * 256:(h + 1) * 256]
                    for t, (src_t, o) in enumerate(
                            ((x8, 0), (x8s, 0), (x8, 2))):
                        nc.tensor.matmul(
                            half,
                            src_t[:, :, off + o:off + o + 128],
                            w8k[:, t, :, :],
                            start=False,
                            stop=(t == 2),
                            perf_mode=mybir.MatmulPerfMode.DoubleRow,
                        )
                # 64*phi = max(raw, min(64*exp(raw/64 + ln64 - 1), 64))
                e = tmp.tile([128, 512], F32, tag="nte", name="e_nt")
                nc.scalar.activation(
                    e, kt_ps, AF.Exp, scale=1.0 / 64.0, bias=bconst[:, 0:1])
                nc.vector.scalar_tensor_tensor(
                    kT[:, ip * 2:ip * 2 + 2, :].rearrange("p i d -> p (i d)"),
                    e, 64.0, kt_ps, ALU.min, ALU.max)
                for ch in range(CT):
                    for ii in (ip * 2, ip * 2 + 1):
                        nc.tensor.matmul(
                            kv_ps[ch],
                            kT[:, ii, ch * 128:(ch + 1) * 128],
                            vT[:, ii, :],
                            start=(ii == 0),
                            stop=(ii == NT - 1),
                        )
            for ch in range(CT):
                nc.vector.tensor_copy(kv_sb[:, ch, :], kv_ps[ch])

            # ---- phase Q: conv q in [c, n] layout ----------------------
            for ct in range(CT):
                bq64 = obk[0:1, 1024 + ct * 128:1024 + (ct + 1) * 128]
                for j in range(NJ):
                    q_ps = ps.tile([128, 512], F32, tag="bank", name="q_ps")
                    nc.tensor.matmul(q_ps, bq64, ones512, start=True, stop=False)
                    for t, (src_t, o) in enumerate(
                            ((x8, 0), (x8s, 0), (x8, 2))):
                        nc.tensor.matmul(
                            q_ps,
                            w8q[:, t, :, ct * 128:(ct + 1) * 128],
                            src_t[:, :, j * 512 + o:j * 512 + o + 512],
                            start=False,
                            stop=(t == 2),
                            perf_mode=mybir.MatmulPerfMode.DoubleRow,
                        )
                    e = tmp.tile([128, 512], F32, tag="qte", name="e_q")
                    nc.scalar.activation(
                        e, q_ps, AF.Exp, scale=1.0 / 64.0, bias=bconst[:, 0:1])
                    nc.vector.scalar_tensor_tensor(
                        qphi[ct][:, j * 512:(j + 1) * 512],
                        e, 64.0, q_ps, ALU.min, ALU.max)

            # ---- phase OUT: g = gelu(q@kv), int8 rows + scales ---------
            os_sb = per.tile([128, CT], F32, tag="os", name="os_sb")
            for dt in range(CT):
                gb = per.tile([128, N], BF16, tag=f"gb{dt}", name=f"gb{dt}")
                for j in range(NJ):
                    o_ps = ps.tile([128, 512], F32, tag="bank", name="o_ps")
                    for ch in range(CT):
                        nc.tensor.matmul(
                            o_ps,
                            kv_sb[:, ch, dt * 128:(dt + 1) * 128],
                            qphi[ch][:, j * 512:(j + 1) * 512],
                            start=(ch == 0),
                            stop=(ch == CT - 1),
                        )
                    nc.scalar.activation(gb[:, j * 512:(j + 1) * 512], o_ps,
                                         AF.Gelu, scale=1.0 / 4096.0)
                gm = tmp.tile([128, 1], F32, tag="gm", name="gm")
                nc.vector.tensor_reduce(gm, gb, mybir.AxisListType.X, ALU.max,
                                        apply_absolute_value=True)
                nc.vector.tensor_scalar(gm, gm, 1e-30, None, ALU.max)
                inv = tmp.tile([128, 1], F32, tag="inv", name="inv")
                nc.vector.reciprocal(inv, gm)
                og = per.tile([128, N], I8, tag=f"og{dt}", name=f"og{dt}")
                nc.vector.tensor_scalar(og, gb, inv[:, 0:1], 127.0,
                                        ALU.mult, ALU.mult)
                nc.vector.tensor_scalar(os_sb[:, dt:dt + 1], gm, 1.0 / 127.0,
                                        None, ALU.mult)
                nc.sync.dma_start(out=og_d[dt * 128:(dt + 1) * 128, :], in_=og)
            nc.sync.dma_start(out=os_d[:, :], in_=os_sb)

    nc.compile()
    return nc


@numba.njit(fastmath=True)
def _quant_half(x3, out3, scale2):
    # per-row absmax -> int8 in [:, :, 1:N+1] (cols 0 / N+1 stay zero padding)
    nb, npart, n = x3.shape
    for b in range(nb):
        for p in range(npart):
            m = 0.0
            for i in range(n):
                m = max(m, abs(x3[b, p, i]))
            m = max(m, 1e-30)
            s = 127.0 / m
            for i in range(n):
                out3[b, p, i + 1] = np.int8(np.rint(x3[b, p, i] * s))
            scale2[b, p] = m * (1.0 / 127.0)


@numba.njit(fastmath=True)
def _dequant_rows(og, osc, x, out):
    rows, n = og.shape
    for r in range(rows):
        s = osc[r]
        for i in range(n):
            out[r, i] = og[r, i] * s + x[r, i]


def _prep_w(conv_w, conv_b):
    conv_w = np.asarray(conv_w, dtype=np.float32)
    conv_b = np.asarray(conv_b, dtype=np.float32)
    # conv weights x64 in fp8, DoubleRow slot layout [p, t, ci, co]
    w = conv_w.transpose(2, 1, 0).reshape(3, CT, 128, 2 * C)
    w = w.transpose(1, 0, 2, 3)                      # [ci, t, p, co]
    w8q = np.ascontiguousarray(
        w[:, :, :, :C].transpose(2, 1, 0, 3) * WS).reshape(128, 6 * 256).astype(F8)
    w8k = np.ascontiguousarray(
        w[:, :, :, C:].transpose(2, 1, 0, 3) * WS).reshape(128, 6 * 256).astype(F8)
    obk = np.ones((1, 1280), dtype=np.float32)
    obk[0, 512:768] = WS * (conv_b[C:] + 1.0)
    obk[0, 768:1024] = WS * (conv_b[C:] + 1.0)
    obk[0, 1024:1280] = WS * (conv_b[:C] + 1.0)
    obk = obk.astype(BF)
    return w8k, w8q, obk


_STATE = None
_XI_BUF = None


def _get_state():
    global _STATE
    if _STATE is None:
        from concourse.bass2jax import (
            _bass_exec_p, install_neuronx_cc_hook, partition_id_tensor)
        from jax.experimental.shard_map import shard_map

        nc = _build_nc()
        install_neuronx_cc_hook()

        partition_name = (nc.partition_id_tensor.name
                          if nc.partition_id_tensor else None)
        in_names, out_names, out_avals = [], [], []
        for alloc in nc.m.functions[0].allocations:
            if not isinstance(alloc, mybir.MemoryLocationSet):
                continue
            name = alloc.memorylocations[0].name
            if alloc.kind == "ExternalInput":
                if name != partition_name:
                    in_names.append(name)
            elif alloc.kind == "ExternalOutput":
                out_names.append(name)
                out_avals.append(jax.core.ShapedArray(
                    tuple(alloc.tensor_shape), mybir.dt.np(alloc.dtype)))
        dbg_zero = {}
        if nc.dbg_addr is not None:
            dbg_zero = {nc.dbg_addr.name: np.zeros((1, 2), np.uint32)}
            if nc.dbg_addr.name not in in_names:
                in_names.append(nc.dbg_addr.name)
        n_params = len(in_names)
        n_outs = len(out_names)
        all_names = in_names + out_names
        if partition_name is not None:
            all_names.append(partition_name)

        def _body(*args):
            operands = list(args)
            if partition_name is not None:
                operands.append(partition_id_tensor())
            return tuple(_bass_exec_p.bind(
                *operands,
                out_avals=tuple(out_avals),
                in_names=tuple(all_names),
                out_names=tuple(out_names),
                lowering_input_output_aliases=(),
                sim_require_finite=True,
                sim_require_nnan=True,
                nc=nc,
            ))

        devices = jax.devices()[:NCORES]
        mesh = Mesh(np.asarray(devices), ("core",))
        sharded = jax.jit(
            shard_map(_body, mesh=mesh,
                      in_specs=(PartitionSpec("core"),) * (n_params + n_outs),
                      out_specs=(PartitionSpec("core"),) * n_outs,
                      check_rep=False),
            donate_argnums=tuple(range(n_params, n_params + n_outs)),
            keep_unused=True,
        )
        # Donated output buffers built on device (uploading host zeros would
        # cost another ~8 MB of wire per call).
        zero_shapes = [(NCORES * a.shape[0], *a.shape[1:]) for a in out_avals]
        zero_dtypes = [a.dtype for a in out_avals]
        sh = NamedSharding(mesh, PartitionSpec("core"))
        zeros_fn = jax.jit(
            lambda: tuple(jnp.zeros(s, d)
                          for s, d in zip(zero_shapes, zero_dtypes)),
            out_shardings=(sh,) * n_outs,
        )
        _STATE = {
            "in_names": in_names,
            "out_names": out_names,
            "sharded": sharded,
            "zeros_fn": zeros_fn,
            "dbg_zero": dbg_zero,
            "sharding": sh,
        }
    return _STATE


def kernel(x: np.ndarray, conv_w: np.ndarray, conv_b: np.ndarray) -> np.ndarray:
    st = _get_state()
    sh = st["sharding"]
    x = np.asarray(x, dtype=np.float32)

    global _XI_BUF
    if _XI_BUF is None:
        _XI_BUF = np.zeros((CT, B, 128, NP), dtype=np.int8)
    xi = _XI_BUF
    rs = np.empty((CT, B, 128), dtype=np.float32)
    x4 = x.reshape(B, CT, 128, N)
    # quantize channel-half 0, start its (async) upload, then quantize
    # half 1 while half 0 is on the ~46 MB/s wire
    xi_dev = []
    for ci in range(CT):
        _quant_half(x4[:, ci], xi[ci], rs[ci])
        xi_dev.append(jax.device_put(xi[ci].reshape(B * 128, NP), sh))
    w8k, w8q, obk = _prep_w(conv_w, conv_b)
    params = {
        "xi0": xi_dev[0],
        "xi1": xi_dev[1],
        "scl": np.ascontiguousarray(rs.transpose(1, 2, 0)).reshape(B * 128, CT),
        # [128, 2, 1536] sliced into 8 x [16, 2, 1536] core shards = itself
        "wsh": np.stack([w8k, w8q], axis=1),
        "obk": np.ascontiguousarray(
            np.broadcast_to(obk, (B, 1, 1280))).reshape(B, 1280),
    }
    for name, z in st["dbg_zero"].items():
        params[name] = np.ascontiguousarray(
            np.broadcast_to(z, (B * z.shape[0], z.shape[1])))
    zeros = st["zeros_fn"]()
    outs = st["sharded"](*[params[n] for n in st["in_names"]], *zeros)
    out_map = dict(zip(st["out_names"], outs))
    for o in outs:
        o.copy_to_host_async()
    # fetch per core shard, dequant + residual each batch while the next
    # shard is still on the wire
    og_shards = sorted(out_map["og"].addressable_shards,
                       key=lambda s: s.index[0].start)
    os_shards = sorted(out_map["os"].addressable_shards,
                       key=lambda s: s.index[0].start)
    out = np.empty((B, C, N), dtype=np.float32)
    for b in range(B):
        og_b = np.asarray(og_shards[b].data)                     # [C, N] int8
        osc_b = np.ascontiguousarray(
            np.asarray(os_shards[b].data).T).reshape(C)          # [C] f32
        _dequant_rows(og_b, osc_b, x[b], out[b])
    return out
